# revision 17
# baseline (speedup 1.0000x reference)
"""Causal self-attention (B=4, T=2048, D=1024, H=16) on 8 trn2 NeuronCores.

Sharding: core c -> (batch b = c // 2, head-group g = c % 2). Each core runs
one batch element with 8 of the 16 heads: column-sharded Wq/Wk/Wv, row-sharded
Wp. Per-core output is a partial product of the output projection; the host
sums the two head-group partials per batch (bp is added on-device by group 0
via a broadcast input; group 1 gets zeros).

v3: on top of v2's software-pipelined schedule:
  - DMA head: x chunk 0 rides the gpsimd queue alone while the weights stream
    on the sync queue in need-order (wq, wk, wv, wp, bpb) -- the first
    projection matmul starts ~9us earlier.
  - Causal mask: a static 128x128 lower-triangle bf16 mask multiplied in by
    the DVE on just the boundary block of each diagonal tile (the i=1 member
    of a diagonal pair also shrinks its scores/exp/AV ranges by 128 cols, so
    no memset is needed). Replaces the per-tile gpsimd affine_select over the
    whole [*, 2, 512-r0] region: shorter exp->AV latency, gpsimd off the
    critical path, ~3us less PE work.
  - Softmax normalization: the denominator spread (r4) DMAs straight out of
    PSUM in parallel with the U copy; the final head-pair skips the copy
    entirely (DVE multiply reads PSUM) since no later AV needs the bank.
  - Filler: one continuous queue (no per-chunk drains) with an adaptive
    pump quota, and the output projections deferred (op0 fills chunk-2,
    op1+op2+v3 fill chunk-3) so the exp-bound last chunk keeps the PE fed.
  - Last chunk's output projection split pr0-2 / pr3: the pr0-2 partials and
    bias accumulate into SBUF while hp3 is still running; only 8 pr=3
    matmuls + adds + stores remain after the last normalization.
All matmul operands are stored bf16 (PSUM accumulation stays fp32).
"""

import numpy as np
import ml_dtypes

import concourse.mybir as mybir
import concourse.tile as tile
from concourse import bacc
from concourse.bass_utils import run_bass_kernel_spmd

B, T, D, H_FULL = 4, 2048, 1024, 16
H = H_FULL // 2          # heads per core
HD = 64                  # head dim
DH = H * HD              # 512, per-core head width
P = 128
TT = T // P              # 16 t tiles
TC = T // 512            # 4 t chunks
KD = D // P              # 8 contraction tiles over D
PR = H // 2              # 4 head pairs
N_CORES = 8

F32 = mybir.dt.float32
BF16 = mybir.dt.bfloat16


class Filler:
    """Queue of deferred PE work (projections / output projection), emitted
    in small bites between attention matmul groups so the in-order PE queue
    always has independent work while ACT runs exp."""

    def __init__(self, total_points):
        self.units = []      # list of (key, generator) pairs
        self.cur = None
        self.cur_key = None
        self.mms = 0         # matmuls remaining (approximate pacing weight)
        self.acc = 0.0
        self.done_keys = set()
        self.points_left = total_points

    def add(self, gen, n_mms, key=None):
        self.units.append((key, gen))
        self.mms += n_mms

    def pump(self, n):
        """Emit work until n matmuls have been issued (or queue empty)."""
        done = 0
        while done < n:
            if self.cur is None:
                if not self.units:
                    return
                self.cur_key, self.cur = self.units.pop(0)
            for kind, thunk in self.cur:
                thunk()
                if kind == "mm":
                    self.mms -= 1
                    done += 1
                    if done >= n:
                        break
            else:
                self.done_keys.add(self.cur_key)
                self.cur = None

    def ensure(self, key):
        """Emit whole units until the unit tagged `key` has been fully
        emitted. Emission order IS program order -- a consumer emitted
        before its producer reads stale data -- so anything an upcoming
        instruction reads must be forced out of the queue first."""
        if key in self.done_keys or not any(
                k == key for k, _ in self.units) and self.cur_key != key:
            return
        while key not in self.done_keys and (self.cur or self.units):
            if self.cur is None:
                self.cur_key, self.cur = self.units.pop(0)
            for kind, thunk in self.cur:
                thunk()
                if kind == "mm":
                    self.mms -= 1
            self.done_keys.add(self.cur_key)
            self.cur = None

    def pump_point(self):
        """One pacing point: emit enough matmuls to cover the exp-vs-PE
        deficit of one tp-pair, scaled up when the queue is deep relative
        to the remaining pacing points (late chunks)."""
        quota = max(4.0, self.mms / max(self.points_left, 1))
        quota = min(quota, 8.0)
        self.points_left -= 1
        self.acc += quota
        n = int(self.acc)
        if n > 0:
            self.acc -= n
            self.pump(n)

    def drain(self):
        self.pump(1 << 30)


def build_nc():
    nc = bacc.Bacc(None, target_bir_lowering=False)

    xt = nc.dram_tensor("xt", [D, T], BF16, kind="ExternalInput")
    wq = nc.dram_tensor("wq", [D, DH], BF16, kind="ExternalInput")
    wk = nc.dram_tensor("wk", [D, DH], BF16, kind="ExternalInput")
    wv = nc.dram_tensor("wv", [D, DH], BF16, kind="ExternalInput")
    bq = nc.dram_tensor("bq", [P, PR], F32, kind="ExternalInput")
    bk = nc.dram_tensor("bk", [P, PR], F32, kind="ExternalInput")
    bvb = nc.dram_tensor("bvb", [P, DH], F32, kind="ExternalInput")
    wp = nc.dram_tensor("wp", [DH, D], BF16, kind="ExternalInput")
    bpb = nc.dram_tensor("bpb", [P, D], F32, kind="ExternalInput")
    y = nc.dram_tensor("y", [T, D], F32, kind="ExternalOutput")

    xt_r = xt.rearrange("(o p) t -> p o t", p=P)
    wq_r = wq.rearrange("(o p) f -> p o f", p=P)
    wk_r = wk.rearrange("(o p) f -> p o f", p=P)
    wv_r = wv.rearrange("(o p) f -> p o f", p=P)

    with tile.TileContext(nc) as tc:
        with (
            tc.tile_pool(name="persist", bufs=1) as pp,
            tc.tile_pool(name="xpool", bufs=2) as xpool,
            tc.tile_pool(name="epool", bufs=6) as epool,
            tc.tile_pool(name="upool", bufs=4) as upool,
            tc.tile_pool(name="rpool", bufs=2) as rpool,
            tc.tile_pool(name="ypool", bufs=3) as ypool,
            tc.tile_pool(name="ypart", bufs=8) as ypartp,
            tc.tile_pool(name="work", bufs=2, space="PSUM") as work,
            tc.tile_pool(name="psS", bufs=2, space="PSUM") as psS,
            tc.tile_pool(name="psU0", bufs=1, space="PSUM") as psU0,
            tc.tile_pool(name="psU1", bufs=1, space="PSUM") as psU1,
        ):
            # x chunk 0 alone on the gpsimd DMA queue; weights queue on sync
            # in need-order so the first Q-proj matmul only waits for
            # x0 + wq (~2MB) instead of the whole 5.5MB input set.
            xt_tiles = {}

            def load_chunk_x(c):
                xt_tiles[c] = xpool.tile([P, KD, 512], BF16, name="xt_c",
                                         tag="xt")
                if c == 0:
                    # quartered so the first projection matmuls (dk 0..1)
                    # can start as soon as the first piece lands
                    for q4 in range(4):
                        nc.gpsimd.dma_start(
                            xt_tiles[0][:, 2 * q4:2 * q4 + 2, :],
                            xt_r[:, 2 * q4:2 * q4 + 2, 0:512])
                    return
                if c == 1:
                    # gate x1 behind phase 2 (reads wv_s, writes into the
                    # x1 region that the real load then overwrites)
                    nc.gpsimd.dma_start(
                        xt_tiles[1][0:1, 0, 0:3], wv_s[0:1, 0, 6:9])
                nc.gpsimd.dma_start(
                    xt_tiles[c][:], xt_r[:, :, c * 512:(c + 1) * 512])

            load_chunk_x(0)

            bq_s = pp.tile([P, PR], F32, name="bq_s")
            nc.sync.dma_start(bq_s[:], bq[:])
            bk_s = pp.tile([P, PR], F32, name="bk_s")
            nc.sync.dma_start(bk_s[:], bk[:])
            bvb_s = pp.tile([P, DH], F32, name="bvb_s")
            nc.sync.dma_start(bvb_s[:], bvb[:])

            wq_s = pp.tile([P, KD, DH], BF16, name="wq_s")
            wk_s = pp.tile([P, KD, DH], BF16, name="wk_s")
            wv_s = pp.tile([P, KD, DH], BF16, name="wv_s")
            wp_s = pp.tile([P, PR, D], BF16, name="wp_s")
            bpb_s = pp.tile([P, D], F32, name="bpb_s")
            # HBM bandwidth is shared round-robin across in-flight transfers,
            # so a flat issue order starves the first-needed data. The loads
            # are split per head-pair column slice (the q/k proj unit for
            # head-pair m reads only cols [m*128,(m+1)*128)) and staged in
            # phases. Phase boundaries are enforced with tiny "canary" DMAs
            # whose DESTINATION overlaps the next phase's tile (a WAW data
            # dependency the scheduler must honor) and whose source is the
            # previous phase's data (so the canary waits for it to land).
            def wslice(dst, srcr, m):
                nc.sync.dma_start(dst[:, :, m * P:(m + 1) * P],
                                  srcr[:, :, m * P:(m + 1) * P])

            # phase 1: x0 (issued above) + wq/wk head-pair 0
            wslice(wq_s, wq_r, 0)
            wslice(wk_s, wk_r, 0)
            # phase 2 canaries: dst overlaps the start of each gated slice
            # (the real load then overwrites the canary bytes), src reads
            # phase-1 data so the canary waits for it to land
            nc.sync.dma_start(wq_s[0:1, 0, P:DH:P],
                              xt_tiles[0][0:1, 0, 0:3])
            nc.sync.dma_start(wk_s[0:1, 0, P:DH:P],
                              xt_tiles[0][0:1, 0, 4:7])
            nc.sync.dma_start(wv_s[0:1, 0, 0:3], xt_tiles[0][0:1, 0, 8:11])
            nc.sync.dma_start(wv_s[0:1, 4, 0:3], xt_tiles[0][0:1, 0, 12:15])
            # wv first: hp0's AV matmuls need V0 before hp1 needs wq/wk m1
            nc.sync.dma_start(wv_s[:, 0:4, :], wv_r[:, 0:4, :])
            nc.sync.dma_start(wv_s[:, 4:8, :], wv_r[:, 4:8, :])
            for m in range(1, 4):
                wslice(wq_s, wq_r, m)
                wslice(wk_s, wk_r, m)
            # phase 3 canaries: gated on wv
            nc.sync.dma_start(wp_s[0:1, 0, 0:3], wv_s[0:1, 0, 3:6])
            nc.sync.dma_start(
                bpb_s.bitcast(mybir.dt.uint8)[0:1, 0:12],
                wv_s.bitcast(mybir.dt.uint8)[0:1, 0:1, 12:24])
            nc.sync.dma_start(wp_s[:], wp.rearrange("(o p) f -> p o f", p=P))
            nc.sync.dma_start(bpb_s[:], bpb[:])

            # per-chunk tensors (separate tiles -> exact dependency tracking
            # so interleaved chunks never falsely serialize); ot additionally
            # per head-pair so the output projection's pr-accumulation chain
            # can start as soon as the first pair is normalized
            qt = [pp.tile([P, PR, 512], BF16, name=f"qt{c}") for c in range(TC)]
            kt = [pp.tile([P, PR, 512], BF16, name=f"kt{c}") for c in range(TC)]
            vv = [pp.tile([P, 4, H, HD + 1], BF16, name=f"vv{c}")
                  for c in range(TC)]
            ot = [[pp.tile([P, 512], BF16, name=f"ot{c}_{pr}")
                   for pr in range(PR)] for c in range(TC)]
            for c in range(TC):
                nc.any.memset(vv[c][:, :, :, HD], 1.0)

            # static lower-triangle mask for the diagonal boundary blocks:
            # tri[p, j, col] = 1 if col >= p else 0 (same for both heads j)
            tri = pp.tile([P, 2, P], BF16, name="tri")
            nc.any.memset(tri[:], 1.0)
            nc.gpsimd.affine_select(
                out=tri[:], in_=tri[:],
                compare_op=mybir.AluOpType.is_ge,
                fill=0.0, base=0, pattern=[[0, 2], [1, P]],
                channel_multiplier=-1,
            )

            def proj_unit_gen(c, kind, m):
                """One projection subunit: 8 accumulating matmuls + bias add.
                kind: 0=Q, 1=K, 2=V(m = t4)."""
                xt_c = xt_tiles[c]
                pq = work.tile([P, 512], F32, name="pq", tag="pp")
                if kind < 2:
                    w_s = (wq_s, wk_s)[kind]
                    for dk in range(KD):
                        yield ("mm", (lambda dk=dk: nc.tensor.matmul(
                            pq[:],
                            w_s[:, dk, m * P:(m + 1) * P],
                            xt_c[:, dk, :],
                            start=(dk == 0),
                            stop=(dk == KD - 1),
                        )))
                    dst = (qt, kt)[kind]
                    b_s = (bq_s, bk_s)[kind]
                    yield ("free", (lambda: nc.vector.tensor_tensor(
                        out=dst[c][:, m, :],
                        in0=pq[:],
                        in1=b_s[:, m, None].to_broadcast((P, 512)),
                        op=mybir.AluOpType.add,
                    )))
                else:
                    for dk in range(KD):
                        yield ("mm", (lambda dk=dk: nc.tensor.matmul(
                            pq[:],
                            xt_c[:, dk, m * P:(m + 1) * P],
                            wv_s[:, dk, :],
                            start=(dk == 0),
                            stop=(dk == KD - 1),
                        )))
                    yield ("free", (lambda: nc.vector.tensor_tensor(
                        out=vv[c][:, m, :, 0:HD],
                        in0=pq.rearrange("p (h d) -> p h d", h=H),
                        in1=bvb_s.rearrange("p (h d) -> p h d", h=H),
                        op=mybir.AluOpType.add,
                    )))

            def outproj_unit_gen(c, tt4, n2):
                """One output-projection subunit: 4 accumulating matmuls +
                bias add + store. tt4 = t-tile within chunk, n2 = D half."""
                tt_ = 4 * c + tt4
                ts_ = slice(tt_ * P, (tt_ + 1) * P)
                ns = slice(n2 * 512, (n2 + 1) * 512)
                py = work.tile([P, 512], F32, name="py", tag="pp")
                for pr in range(PR):
                    yield ("mm", (lambda pr=pr: nc.tensor.matmul(
                        py[:],
                        ot[c][pr][:, tt4 * P:(tt4 + 1) * P],
                        wp_s[:, pr, ns],
                        start=(pr == 0),
                        stop=(pr == PR - 1),
                    )))
                yt = ypool.tile([P, 512], F32, name="yt", tag="yt")
                yield ("free", (lambda: nc.vector.tensor_tensor(
                    out=yt[:], in0=py[:], in1=bpb_s[:, ns],
                    op=mybir.AluOpType.add,
                )))
                yield ("free", (lambda: nc.gpsimd.dma_start(y[ts_, ns],
                                                            yt[:])))

            # last-chunk output projection, split so only the pr=3 matmul
            # trails the final normalization
            ypart_tiles = {}

            def outproj_partial_gen(c, tt4, n2):
                """pr 0..2 accumulation + bias -> fp32 SBUF partial."""
                ns = slice(n2 * 512, (n2 + 1) * 512)
                py = work.tile([P, 512], F32, name="py", tag="pp")
                for pr in range(PR - 1):
                    yield ("mm", (lambda pr=pr: nc.tensor.matmul(
                        py[:],
                        ot[c][pr][:, tt4 * P:(tt4 + 1) * P],
                        wp_s[:, pr, ns],
                        start=(pr == 0),
                        stop=(pr == PR - 2),
                    )))
                yp = ypartp.tile([P, 512], F32, name="yp", tag="yp")
                ypart_tiles[(tt4, n2)] = yp
                yield ("free", (lambda: nc.vector.tensor_tensor(
                    out=yp[:], in0=py[:], in1=bpb_s[:, ns],
                    op=mybir.AluOpType.add,
                )))

            def outproj_tail(c, tt4, n2):
                tt_ = 4 * c + tt4
                ts_ = slice(tt_ * P, (tt_ + 1) * P)
                ns = slice(n2 * 512, (n2 + 1) * 512)
                py = work.tile([P, 512], F32, name="py2", tag="pp")
                nc.tensor.matmul(
                    py[:],
                    ot[c][PR - 1][:, tt4 * P:(tt4 + 1) * P],
                    wp_s[:, PR - 1, ns],
                    start=True, stop=True,
                )
                yt = ypool.tile([P, 512], F32, name="yt", tag="yt")
                nc.vector.tensor_tensor(
                    out=yt[:], in0=py[:], in1=ypart_tiles[(tt4, n2)][:],
                    op=mybir.AluOpType.add,
                )
                nc.sync.dma_start(y[ts_, ns], yt[:])

            def add_proj_qk(fil, c):
                if c not in xt_tiles:
                    load_chunk_x(c)
                for m in range(4):
                    for kind in range(2):
                        fil.add(proj_unit_gen(c, kind, m), KD,
                                key=("qk", c, kind, m))

            def add_proj_v(fil, c):
                for m in range(4):
                    fil.add(proj_unit_gen(c, 2, m), KD, key=("v", c, m))

            def add_outproj(fil, c):
                for tt4 in range(4):
                    for n2 in range(2):
                        fil.add(outproj_unit_gen(c, tt4, n2), PR)

            def attn_chunk(c, fil, after_hp=None):
                ntk = 4 * c + 4
                for hp in range(PR):
                    # program order is emission order: this head-pair's q/k
                    # projection units (hp-major FIFO order, so ensuring the
                    # k unit flushes the q unit too) must be emitted before
                    # its first scores matmul
                    fil.ensure(("qk", c, 1, hp))
                    ups = [
                        (psU0 if j == 0 else psU1).tile(
                            [HD + 1, 512], F32, name=f"up{j}", tag=f"u{j}")
                        for j in (0, 1)
                    ]
                    for tp in range(0, ntk, 2):
                        diag = tp >= 4 * c
                        r0 = P * (tp - 4 * c) if diag else 0
                        sps, ets = [], []
                        for i in (0, 1):
                            sps.append(psS.tile(
                                [P, 2, 512], F32, name="sp", tag="s"))
                            ets.append(epool.tile(
                                [P, 2, 512], BF16, name="et", tag="e"))
                        for i in (0, 1):
                            t = tp + i
                            tc_, t4 = t // 4, t % 4
                            ri = r0 + P * i if diag else 0
                            for j in (0, 1):
                                # j=0 rows 0-63, j=1 rows 64-127: disjoint
                                # row groups run concurrently on the PE
                                pb = 64 * j
                                nc.tensor.matmul(
                                    sps[i][:, j, ri:512],
                                    kt[tc_][pb:pb + 64, hp,
                                            t4 * P:(t4 + 1) * P],
                                    qt[c][pb:pb + 64, hp, ri:512],
                                    start=True,
                                    stop=True,
                                )
                        for i in (0, 1):
                            ri = r0 + P * i if diag else 0
                            nc.scalar.activation(
                                ets[i][:, :, ri:512], sps[i][:, :, ri:512],
                                mybir.ActivationFunctionType.Exp,
                                scale=float(1.0 / np.sqrt(HD)),
                            )
                            if diag:
                                # zero the upper triangle of the boundary
                                # 128-block with a static-mask DVE multiply
                                # (same mask for both heads)
                                nc.vector.tensor_tensor(
                                    out=ets[i][:, :, ri:ri + P],
                                    in0=ets[i][:, :, ri:ri + P],
                                    in1=tri[:],
                                    op=mybir.AluOpType.mult,
                                )
                        # filler between scores/exp and the dependent AV
                        # matmuls: the PE would otherwise stall here
                        fil.pump_point()
                        for i in (0, 1):
                            # producers of vv must be emitted before the AV
                            # matmuls that read them (emission order is
                            # program order)
                            t = tp + i
                            fil.ensure(("v", t // 4, t % 4))
                        for i in (0, 1):
                            t = tp + i
                            tc_, t4 = t // 4, t % 4
                            ri = r0 + P * i if diag else 0
                            for j in (0, 1):
                                nc.tensor.matmul(
                                    ups[j][:, ri:512],
                                    vv[tc_][:, t4, 2 * hp + j, :],
                                    ets[i][:, j, ri:512],
                                    start=(t == 0),
                                    stop=(t == ntk - 1),
                                )
                    # softmax normalization. The reciprocal of the denominator
                    # row is computed via the DMA-spread trick; the spread
                    # reads straight from PSUM so it runs concurrently with
                    # the U copy. The last head-pair of the last chunk skips
                    # the copy (multiply reads PSUM) -- nothing needs the
                    # banks afterwards.
                    lp = nc.allow_low_precision(
                        reason="bf16 softmax normalization; rel tol 2e-2")
                    lp.__enter__()
                    for j in (0, 1):
                        uu = upool.tile([HD + 1, 512], BF16, name="uu",
                                        tag=f"uu{j}")
                        # denominator row first: the spread DMA only waits on
                        # this small copy, not the full-U cast
                        nc.vector.tensor_copy(uu[HD:HD + 1, :],
                                              ups[j][HD:HD + 1, :])
                        r4 = rpool.tile([32, 16], BF16, name="r4", tag="r4")
                        nc.sync.dma_start(r4[:], uu[HD:HD + 1, :])
                        nc.vector.tensor_copy(uu[0:HD, :], ups[j][0:HD, :])
                        usrc = uu
                        r4r = rpool.tile([32, 16], BF16, name="r4r",
                                         tag="r4r")
                        nc.vector.reciprocal(r4r[:], r4[:])
                        rb = rpool.tile([1, 512], BF16, name="rb", tag="rb")
                        nc.sync.dma_start(rb[:], r4r[:])
                        bc = rpool.tile([64, 512], BF16, name="bc",
                                        tag=f"bc{j}")
                        nc.gpsimd.partition_broadcast(bc[:], rb[0:1, :])
                        # normalize on gpsimd (SBUF-only operands): right
                        # behind the broadcast on the same queue, and it
                        # keeps the DVE free for projection bias-adds whose
                        # PSUM-pool reuse gates the PE
                        if j == 0:
                            nc.gpsimd.tensor_tensor(
                                out=ot[c][hp][0:64, :], in0=usrc[0:64, :],
                                in1=bc[:], op=mybir.AluOpType.mult,
                            )
                        else:
                            om = rpool.tile([64, 512], BF16, name="om",
                                            tag="om")
                            nc.gpsimd.tensor_tensor(
                                out=om[:], in0=usrc[0:64, :], in1=bc[:],
                                op=mybir.AluOpType.mult,
                            )
                            nc.sync.dma_start(ot[c][hp][64:128, :], om[:])
                    lp.__exit__(None, None, None)
                    if after_hp is not None:
                        after_hp(hp)

            # ---------------- schedule ----------------
            # chunk-0 Q/K projections run up front (attention needs them);
            # everything else flows through one continuous filler queue:
            #   during chunk 0: v0, qk1, v1
            #   during chunk 1: + qk2, v2
            #   during chunk 2: + qk3, op0
            #   during chunk 3: + v3, op1, op2 (+ op3 partials after hp2)
            # The adaptive pump quota leaves enough for the exp-bound late
            # chunks; the dependency-driven scheduler tolerates any slack.
            n_points = sum(4 * (2 * c + 2) for c in range(TC))
            fil = Filler(n_points)
            # hp0's q/k first, then v0 (hp0's AV ensures pull v0 through the
            # FIFO, so it must sit ahead of the later head-pairs' q/k)
            fil.add(proj_unit_gen(0, 0, 0), KD, key=("qk", 0, 0, 0))
            fil.add(proj_unit_gen(0, 1, 0), KD, key=("qk", 0, 1, 0))
            add_proj_v(fil, 0)
            for m in range(1, 4):
                fil.add(proj_unit_gen(0, 0, m), KD, key=("qk", 0, 0, m))
                fil.add(proj_unit_gen(0, 1, m), KD, key=("qk", 0, 1, m))
            add_proj_qk(fil, 1)
            add_proj_v(fil, 1)
            attn_chunk(0, fil)
            add_proj_qk(fil, 2)
            add_proj_v(fil, 2)
            attn_chunk(1, fil)
            add_proj_qk(fil, 3)
            add_outproj(fil, 0)
            attn_chunk(2, fil)
            add_proj_v(fil, 3)
            add_outproj(fil, 1)

            def after_hp3(hp):
                if hp == PR - 2:
                    for tt4 in range(4):
                        for n2 in range(2):
                            fil.add(outproj_partial_gen(TC - 1, tt4, n2),
                                    PR - 1)
                if hp == PR - 1:
                    # reserve: op2 emits right after the last AV group, so
                    # the PE grinds through it while the final normalization
                    # chain (cast/spread/recip/broadcast) runs on the other
                    # engines
                    add_outproj(fil, 2)

            attn_chunk(3, fil, after_hp=after_hp3)
            fil.drain()
            for tt4 in range(4):
                for n2 in range(2):
                    outproj_tail(TC - 1, tt4, n2)

    nc.compile()
    return nc


_NC_CACHE = None


def _get_nc():
    global _NC_CACHE
    if _NC_CACHE is None:
        _NC_CACHE = build_nc()
    return _NC_CACHE


def _shard_inputs(x, Wq, bq, Wk, bk, Wv, bv, Wp, bp):
    """Build the 8 per-core input maps."""
    bf16 = ml_dtypes.bfloat16
    x = np.asarray(x, dtype=np.float32)
    ca = np.ascontiguousarray
    in_maps = []
    for core in range(N_CORES):
        b, g = core // 2, core % 2
        cols = slice(g * DH, (g + 1) * DH)
        bq_g = np.asarray(bq[cols], np.float32).reshape(PR, P).T
        bk_g = np.asarray(bk[cols], np.float32).reshape(PR, P).T
        bv_g = np.broadcast_to(np.asarray(bv[cols], np.float32), (P, DH))
        if g == 0:
            bp_b = np.broadcast_to(np.asarray(bp, np.float32), (P, D))
        else:
            bp_b = np.zeros((P, D), np.float32)
        in_maps.append({
            "xt": ca(x[b].T.astype(bf16)),
            "wq": ca(np.asarray(Wq, np.float32)[:, cols].astype(bf16)),
            "wk": ca(np.asarray(Wk, np.float32)[:, cols].astype(bf16)),
            "wv": ca(np.asarray(Wv, np.float32)[:, cols].astype(bf16)),
            "bq": ca(bq_g),
            "bk": ca(bk_g),
            "bvb": ca(bv_g),
            "wp": ca(np.asarray(Wp, np.float32)[cols, :].astype(bf16)),
            "bpb": ca(bp_b),
        })
    return in_maps


def run_sharded(inputs, trace=False):
    """Run on 8 cores; returns (full_output, BassKernelResults)."""
    nc = _get_nc()
    in_maps = _shard_inputs(**inputs)
    res = run_bass_kernel_spmd(
        nc, in_maps, core_ids=list(range(N_CORES)), trace=trace
    )
    out = np.empty((B, T, D), np.float32)
    for b in range(B):
        out[b] = res.results[2 * b]["y"] + res.results[2 * b + 1]["y"]
    return out, res


def kernel(**inputs) -> np.ndarray:
    out, _ = run_sharded(inputs)
    return out


# revision 18
# speedup vs baseline: 2.2965x; 2.2965x over previous
"""Causal self-attention (B=4, T=2048, D=1024, H=16) on 8 trn2 NeuronCores.

Sharding: core c -> (batch b = c // 2, head-group g = c % 2). Each core runs
one batch element with 8 of the 16 heads: column-sharded Wq/Wk/Wv, row-sharded
Wp. Per-core output is a partial product of the output projection; the host
sums the two head-group partials per batch (bp is added on-device by group 0
via a broadcast input; group 1 gets zeros).

v3: on top of v2's software-pipelined schedule:
  - DMA head: x chunk 0 rides the gpsimd queue alone while the weights stream
    on the sync queue in need-order (wq, wk, wv, wp, bpb) -- the first
    projection matmul starts ~9us earlier.
  - Causal mask: a static 128x128 lower-triangle bf16 mask multiplied in by
    the DVE on just the boundary block of each diagonal tile (the i=1 member
    of a diagonal pair also shrinks its scores/exp/AV ranges by 128 cols, so
    no memset is needed). Replaces the per-tile gpsimd affine_select over the
    whole [*, 2, 512-r0] region: shorter exp->AV latency, gpsimd off the
    critical path, ~3us less PE work.
  - Softmax normalization: the denominator spread (r4) DMAs straight out of
    PSUM in parallel with the U copy; the final head-pair skips the copy
    entirely (DVE multiply reads PSUM) since no later AV needs the bank.
  - Filler: one continuous queue (no per-chunk drains) with an adaptive
    pump quota, and the output projections deferred (op0 fills chunk-2,
    op1+op2+v3 fill chunk-3) so the exp-bound last chunk keeps the PE fed.
  - Last chunk's output projection split pr0-2 / pr3: the pr0-2 partials and
    bias accumulate into SBUF while hp3 is still running; only 8 pr=3
    matmuls + adds + stores remain after the last normalization.
All matmul operands are stored bf16 (PSUM accumulation stays fp32).
"""

import numpy as np
import ml_dtypes

import concourse.mybir as mybir
import concourse.tile as tile
from concourse import bacc
from concourse.bass_utils import run_bass_kernel_spmd

B, T, D, H_FULL = 4, 2048, 1024, 16
H = H_FULL // 2          # heads per core
HD = 64                  # head dim
DH = H * HD              # 512, per-core head width
P = 128
TT = T // P              # 16 t tiles
TC = T // 512            # 4 t chunks
KD = D // P              # 8 contraction tiles over D
PR = H // 2              # 4 head pairs
N_CORES = 8

F32 = mybir.dt.float32
BF16 = mybir.dt.bfloat16


class Filler:
    """Queue of deferred PE work (projections / output projection), emitted
    in small bites between attention matmul groups so the in-order PE queue
    always has independent work while ACT runs exp."""

    def __init__(self, total_points):
        self.units = []      # list of (key, generator) pairs
        self.cur = None
        self.cur_key = None
        self.mms = 0         # matmuls remaining (approximate pacing weight)
        self.acc = 0.0
        self.done_keys = set()
        self.points_left = total_points

    def add(self, gen, n_mms, key=None):
        self.units.append((key, gen))
        self.mms += n_mms

    def pump(self, n):
        """Emit work until n matmuls have been issued (or queue empty)."""
        done = 0
        while done < n:
            if self.cur is None:
                if not self.units:
                    return
                self.cur_key, self.cur = self.units.pop(0)
            for kind, thunk in self.cur:
                thunk()
                if kind == "mm":
                    self.mms -= 1
                    done += 1
                    if done >= n:
                        break
            else:
                self.done_keys.add(self.cur_key)
                self.cur = None

    def ensure(self, key):
        """Emit whole units until the unit tagged `key` has been fully
        emitted. Emission order IS program order -- a consumer emitted
        before its producer reads stale data -- so anything an upcoming
        instruction reads must be forced out of the queue first."""
        if key in self.done_keys or not any(
                k == key for k, _ in self.units) and self.cur_key != key:
            return
        while key not in self.done_keys and (self.cur or self.units):
            if self.cur is None:
                self.cur_key, self.cur = self.units.pop(0)
            for kind, thunk in self.cur:
                thunk()
                if kind == "mm":
                    self.mms -= 1
            self.done_keys.add(self.cur_key)
            self.cur = None

    def pump_point(self):
        """One pacing point: emit enough matmuls to cover the exp-vs-PE
        deficit of one tp-pair, scaled up when the queue is deep relative
        to the remaining pacing points (late chunks)."""
        quota = max(4.0, self.mms / max(self.points_left, 1))
        quota = min(quota, 8.0)
        self.points_left -= 1
        self.acc += quota
        n = int(self.acc)
        if n > 0:
            self.acc -= n
            self.pump(n)

    def drain(self):
        self.pump(1 << 30)


def build_nc():
    nc = bacc.Bacc(None, target_bir_lowering=False)

    xt = nc.dram_tensor("xt", [D, T], BF16, kind="ExternalInput")
    wq = nc.dram_tensor("wq", [D, DH], BF16, kind="ExternalInput")
    wk = nc.dram_tensor("wk", [D, DH], BF16, kind="ExternalInput")
    wv = nc.dram_tensor("wv", [D, DH], BF16, kind="ExternalInput")
    bq = nc.dram_tensor("bq", [P, PR], F32, kind="ExternalInput")
    bk = nc.dram_tensor("bk", [P, PR], F32, kind="ExternalInput")
    bvb = nc.dram_tensor("bvb", [P, DH], F32, kind="ExternalInput")
    wp = nc.dram_tensor("wp", [DH, D], BF16, kind="ExternalInput")
    bpb = nc.dram_tensor("bpb", [P, D], F32, kind="ExternalInput")
    y = nc.dram_tensor("y", [T, D], F32, kind="ExternalOutput")

    xt_r = xt.rearrange("(o p) t -> p o t", p=P)
    wq_r = wq.rearrange("(o p) f -> p o f", p=P)
    wk_r = wk.rearrange("(o p) f -> p o f", p=P)
    wv_r = wv.rearrange("(o p) f -> p o f", p=P)

    with tile.TileContext(nc) as tc:
        with (
            tc.tile_pool(name="persist", bufs=1) as pp,
            tc.tile_pool(name="xpool", bufs=2) as xpool,
            tc.tile_pool(name="epool", bufs=6) as epool,
            tc.tile_pool(name="upool", bufs=4) as upool,
            tc.tile_pool(name="rpool", bufs=2) as rpool,
            tc.tile_pool(name="ypool", bufs=3) as ypool,
            tc.tile_pool(name="ypart", bufs=8) as ypartp,
            tc.tile_pool(name="work", bufs=2, space="PSUM") as work,
            tc.tile_pool(name="psS", bufs=2, space="PSUM") as psS,
            tc.tile_pool(name="psU0", bufs=1, space="PSUM") as psU0,
            tc.tile_pool(name="psU1", bufs=1, space="PSUM") as psU1,
        ):
            # x chunk 0 alone on the gpsimd DMA queue; weights queue on sync
            # in need-order so the first Q-proj matmul only waits for
            # x0 + wq (~2MB) instead of the whole 5.5MB input set.
            xt_tiles = {}

            def load_chunk_x(c):
                xt_tiles[c] = xpool.tile([P, KD, 512], BF16, name="xt_c",
                                         tag="xt")
                if c == 0:
                    # quartered so the first projection matmuls (dk 0..1)
                    # can start as soon as the first piece lands
                    for q4 in range(4):
                        nc.gpsimd.dma_start(
                            xt_tiles[0][:, 2 * q4:2 * q4 + 2, :],
                            xt_r[:, 2 * q4:2 * q4 + 2, 0:512])
                    return
                if c == 1:
                    # gate x1 behind phase 2 (reads wv_s, writes into the
                    # x1 region that the real load then overwrites)
                    nc.gpsimd.dma_start(
                        xt_tiles[1][0:1, 0, 0:3], wv_s[0:1, 0, 6:9])
                nc.gpsimd.dma_start(
                    xt_tiles[c][:], xt_r[:, :, c * 512:(c + 1) * 512])

            load_chunk_x(0)

            bq_s = pp.tile([P, PR], F32, name="bq_s")
            nc.sync.dma_start(bq_s[:], bq[:])
            bk_s = pp.tile([P, PR], F32, name="bk_s")
            nc.sync.dma_start(bk_s[:], bk[:])
            bvb_s = pp.tile([P, DH], F32, name="bvb_s")
            nc.sync.dma_start(bvb_s[:], bvb[:])

            wq_s = pp.tile([P, KD, DH], BF16, name="wq_s")
            wk_s = pp.tile([P, KD, DH], BF16, name="wk_s")
            wv_s = pp.tile([P, KD, DH], BF16, name="wv_s")
            wp_s = pp.tile([P, PR, D], BF16, name="wp_s")
            bpb_s = pp.tile([P, D], F32, name="bpb_s")
            # HBM bandwidth is shared round-robin across in-flight transfers,
            # so a flat issue order starves the first-needed data. The loads
            # are split per head-pair column slice (the q/k proj unit for
            # head-pair m reads only cols [m*128,(m+1)*128)) and staged in
            # phases. Phase boundaries are enforced with tiny "canary" DMAs
            # whose DESTINATION overlaps the next phase's tile (a WAW data
            # dependency the scheduler must honor) and whose source is the
            # previous phase's data (so the canary waits for it to land).
            def wslice(dst, srcr, m):
                nc.sync.dma_start(dst[:, :, m * P:(m + 1) * P],
                                  srcr[:, :, m * P:(m + 1) * P])

            # phase 1: x0 (issued above) + wq/wk head-pair 0
            wslice(wq_s, wq_r, 0)
            wslice(wk_s, wk_r, 0)
            # phase 2 canaries: dst overlaps the start of each gated slice
            # (the real load then overwrites the canary bytes), src reads
            # phase-1 data so the canary waits for it to land
            nc.sync.dma_start(wq_s[0:1, 0, P:DH:P],
                              xt_tiles[0][0:1, 0, 0:3])
            nc.sync.dma_start(wk_s[0:1, 0, P:DH:P],
                              xt_tiles[0][0:1, 0, 4:7])
            nc.sync.dma_start(wv_s[0:1, 0, 0:3], xt_tiles[0][0:1, 0, 8:11])
            nc.sync.dma_start(wv_s[0:1, 4, 0:3], xt_tiles[0][0:1, 0, 12:15])
            # wv first: hp0's AV matmuls need V0 before hp1 needs wq/wk m1
            nc.sync.dma_start(wv_s[:, 0:4, :], wv_r[:, 0:4, :])
            nc.sync.dma_start(wv_s[:, 4:8, :], wv_r[:, 4:8, :])
            for m in range(1, 4):
                wslice(wq_s, wq_r, m)
                wslice(wk_s, wk_r, m)
            # phase 3 canaries: gated on wv
            nc.sync.dma_start(wp_s[0:1, 0, 0:3], wv_s[0:1, 0, 3:6])
            nc.sync.dma_start(
                bpb_s.bitcast(mybir.dt.uint8)[0:1, 0:12],
                wv_s.bitcast(mybir.dt.uint8)[0:1, 0:1, 12:24])
            nc.sync.dma_start(wp_s[:], wp.rearrange("(o p) f -> p o f", p=P))
            nc.sync.dma_start(bpb_s[:], bpb[:])

            # per-chunk tensors (separate tiles -> exact dependency tracking
            # so interleaved chunks never falsely serialize); ot additionally
            # per head-pair so the output projection's pr-accumulation chain
            # can start as soon as the first pair is normalized
            qt = [pp.tile([P, PR, 512], BF16, name=f"qt{c}") for c in range(TC)]
            kt = [pp.tile([P, PR, 512], BF16, name=f"kt{c}") for c in range(TC)]
            vv = [pp.tile([P, 4, H, HD + 1], BF16, name=f"vv{c}")
                  for c in range(TC)]
            ot = [[pp.tile([P, 512], BF16, name=f"ot{c}_{pr}")
                   for pr in range(PR)] for c in range(TC)]
            for c in range(TC):
                nc.any.memset(vv[c][:, :, :, HD], 1.0)

            # static lower-triangle mask for the diagonal boundary blocks:
            # tri[p, j, col] = 1 if col >= p else 0 (same for both heads j)
            tri = pp.tile([P, 2, P], BF16, name="tri")
            nc.any.memset(tri[:], 1.0)
            nc.gpsimd.affine_select(
                out=tri[:], in_=tri[:],
                compare_op=mybir.AluOpType.is_ge,
                fill=0.0, base=0, pattern=[[0, 2], [1, P]],
                channel_multiplier=-1,
            )

            def proj_unit_gen(c, kind, m):
                """One projection subunit: 8 accumulating matmuls + bias add.
                kind: 0=Q, 1=K, 2=V(m = t4)."""
                xt_c = xt_tiles[c]
                pq = work.tile([P, 512], F32, name="pq", tag="pp")
                if kind < 2:
                    w_s = (wq_s, wk_s)[kind]
                    for dk in range(KD):
                        yield ("mm", (lambda dk=dk: nc.tensor.matmul(
                            pq[:],
                            w_s[:, dk, m * P:(m + 1) * P],
                            xt_c[:, dk, :],
                            start=(dk == 0),
                            stop=(dk == KD - 1),
                        )))
                    dst = (qt, kt)[kind]
                    b_s = (bq_s, bk_s)[kind]
                    yield ("free", (lambda: nc.vector.tensor_tensor(
                        out=dst[c][:, m, :],
                        in0=pq[:],
                        in1=b_s[:, m, None].to_broadcast((P, 512)),
                        op=mybir.AluOpType.add,
                    )))
                else:
                    for dk in range(KD):
                        yield ("mm", (lambda dk=dk: nc.tensor.matmul(
                            pq[:],
                            xt_c[:, dk, m * P:(m + 1) * P],
                            wv_s[:, dk, :],
                            start=(dk == 0),
                            stop=(dk == KD - 1),
                        )))
                    yield ("free", (lambda: nc.vector.tensor_tensor(
                        out=vv[c][:, m, :, 0:HD],
                        in0=pq.rearrange("p (h d) -> p h d", h=H),
                        in1=bvb_s.rearrange("p (h d) -> p h d", h=H),
                        op=mybir.AluOpType.add,
                    )))

            def outproj_unit_gen(c, tt4, n2):
                """One output-projection subunit: 4 accumulating matmuls +
                bias add + store. tt4 = t-tile within chunk, n2 = D half."""
                tt_ = 4 * c + tt4
                ts_ = slice(tt_ * P, (tt_ + 1) * P)
                ns = slice(n2 * 512, (n2 + 1) * 512)
                py = work.tile([P, 512], F32, name="py", tag="pp")
                for pr in range(PR):
                    yield ("mm", (lambda pr=pr: nc.tensor.matmul(
                        py[:],
                        ot[c][pr][:, tt4 * P:(tt4 + 1) * P],
                        wp_s[:, pr, ns],
                        start=(pr == 0),
                        stop=(pr == PR - 1),
                    )))
                yt = ypool.tile([P, 512], F32, name="yt", tag="yt")
                yield ("free", (lambda: nc.vector.tensor_tensor(
                    out=yt[:], in0=py[:], in1=bpb_s[:, ns],
                    op=mybir.AluOpType.add,
                )))
                yield ("free", (lambda: nc.gpsimd.dma_start(y[ts_, ns],
                                                            yt[:])))

            # last-chunk output projection, split so only the pr=3 matmul
            # trails the final normalization
            ypart_tiles = {}

            def outproj_partial_gen(c, tt4, n2):
                """pr 0..2 accumulation + bias -> fp32 SBUF partial."""
                ns = slice(n2 * 512, (n2 + 1) * 512)
                py = work.tile([P, 512], F32, name="py", tag="pp")
                for pr in range(PR - 1):
                    yield ("mm", (lambda pr=pr: nc.tensor.matmul(
                        py[:],
                        ot[c][pr][:, tt4 * P:(tt4 + 1) * P],
                        wp_s[:, pr, ns],
                        start=(pr == 0),
                        stop=(pr == PR - 2),
                    )))
                yp = ypartp.tile([P, 512], F32, name="yp", tag="yp")
                ypart_tiles[(tt4, n2)] = yp
                yield ("free", (lambda: nc.vector.tensor_tensor(
                    out=yp[:], in0=py[:], in1=bpb_s[:, ns],
                    op=mybir.AluOpType.add,
                )))

            def outproj_tail(c, tt4, n2):
                tt_ = 4 * c + tt4
                ts_ = slice(tt_ * P, (tt_ + 1) * P)
                ns = slice(n2 * 512, (n2 + 1) * 512)
                py = work.tile([P, 512], F32, name="py2", tag="pp")
                nc.tensor.matmul(
                    py[:],
                    ot[c][PR - 1][:, tt4 * P:(tt4 + 1) * P],
                    wp_s[:, PR - 1, ns],
                    start=True, stop=True,
                )
                yt = ypool.tile([P, 512], F32, name="yt", tag="yt")
                nc.vector.tensor_tensor(
                    out=yt[:], in0=py[:], in1=ypart_tiles[(tt4, n2)][:],
                    op=mybir.AluOpType.add,
                )
                nc.sync.dma_start(y[ts_, ns], yt[:])

            def add_proj_qk(fil, c):
                if c not in xt_tiles:
                    load_chunk_x(c)
                for m in range(4):
                    for kind in range(2):
                        fil.add(proj_unit_gen(c, kind, m), KD,
                                key=("qk", c, kind, m))

            def add_proj_v(fil, c):
                for m in range(4):
                    fil.add(proj_unit_gen(c, 2, m), KD, key=("v", c, m))

            def add_outproj(fil, c):
                for tt4 in range(4):
                    for n2 in range(2):
                        fil.add(outproj_unit_gen(c, tt4, n2), PR)

            def attn_chunk(c, fil, after_hp=None):
                ntk = 4 * c + 4
                for hp in range(PR):
                    # program order is emission order: this head-pair's q/k
                    # projection units (hp-major FIFO order, so ensuring the
                    # k unit flushes the q unit too) must be emitted before
                    # its first scores matmul
                    fil.ensure(("qk", c, 1, hp))
                    ups = [
                        (psU0 if j == 0 else psU1).tile(
                            [HD + 1, 512], F32, name=f"up{j}", tag=f"u{j}")
                        for j in (0, 1)
                    ]
                    for tp in range(0, ntk, 2):
                        diag = tp >= 4 * c
                        r0 = P * (tp - 4 * c) if diag else 0
                        sps, ets = [], []
                        for i in (0, 1):
                            sps.append(psS.tile(
                                [P, 2, 512], F32, name="sp", tag="s"))
                            ets.append(epool.tile(
                                [P, 2, 512], BF16, name="et", tag="e"))
                        for i in (0, 1):
                            t = tp + i
                            tc_, t4 = t // 4, t % 4
                            ri = r0 + P * i if diag else 0
                            for j in (0, 1):
                                # j=0 rows 0-63, j=1 rows 64-127: disjoint
                                # row groups run concurrently on the PE
                                pb = 64 * j
                                nc.tensor.matmul(
                                    sps[i][:, j, ri:512],
                                    kt[tc_][pb:pb + 64, hp,
                                            t4 * P:(t4 + 1) * P],
                                    qt[c][pb:pb + 64, hp, ri:512],
                                    start=True,
                                    stop=True,
                                )
                        for i in (0, 1):
                            ri = r0 + P * i if diag else 0
                            nc.scalar.activation(
                                ets[i][:, :, ri:512], sps[i][:, :, ri:512],
                                mybir.ActivationFunctionType.Exp,
                                scale=float(1.0 / np.sqrt(HD)),
                            )
                            if diag:
                                # zero the upper triangle of the boundary
                                # 128-block with a static-mask DVE multiply
                                # (same mask for both heads)
                                nc.vector.tensor_tensor(
                                    out=ets[i][:, :, ri:ri + P],
                                    in0=ets[i][:, :, ri:ri + P],
                                    in1=tri[:],
                                    op=mybir.AluOpType.mult,
                                )
                        # filler between scores/exp and the dependent AV
                        # matmuls: the PE would otherwise stall here
                        fil.pump_point()
                        for i in (0, 1):
                            # producers of vv must be emitted before the AV
                            # matmuls that read them (emission order is
                            # program order)
                            t = tp + i
                            fil.ensure(("v", t // 4, t % 4))
                        for i in (0, 1):
                            t = tp + i
                            tc_, t4 = t // 4, t % 4
                            ri = r0 + P * i if diag else 0
                            for j in (0, 1):
                                nc.tensor.matmul(
                                    ups[j][:, ri:512],
                                    vv[tc_][:, t4, 2 * hp + j, :],
                                    ets[i][:, j, ri:512],
                                    start=(t == 0),
                                    stop=(t == ntk - 1),
                                )
                    # softmax normalization. The reciprocal of the denominator
                    # row is computed via the DMA-spread trick; the spread
                    # reads straight from PSUM so it runs concurrently with
                    # the U copy. The last head-pair of the last chunk skips
                    # the copy (multiply reads PSUM) -- nothing needs the
                    # banks afterwards.
                    lp = nc.allow_low_precision(
                        reason="bf16 softmax normalization; rel tol 2e-2")
                    lp.__enter__()
                    for j in (0, 1):
                        uu = upool.tile([HD + 1, 512], BF16, name="uu",
                                        tag=f"uu{j}")
                        # denominator row first: the spread DMA only waits on
                        # this small copy, not the full-U cast
                        nc.vector.tensor_copy(uu[HD:HD + 1, :],
                                              ups[j][HD:HD + 1, :])
                        r4 = rpool.tile([32, 16], BF16, name="r4", tag="r4")
                        nc.sync.dma_start(r4[:], uu[HD:HD + 1, :])
                        nc.vector.tensor_copy(uu[0:HD, :], ups[j][0:HD, :])
                        usrc = uu
                        r4r = rpool.tile([32, 16], BF16, name="r4r",
                                         tag="r4r")
                        nc.vector.reciprocal(r4r[:], r4[:])
                        rb = rpool.tile([1, 512], BF16, name="rb", tag="rb")
                        nc.sync.dma_start(rb[:], r4r[:])
                        bc = rpool.tile([64, 512], BF16, name="bc",
                                        tag=f"bc{j}")
                        nc.gpsimd.partition_broadcast(bc[:], rb[0:1, :])
                        if j == 0:
                            nc.vector.tensor_tensor(
                                out=ot[c][hp][0:64, :], in0=usrc[0:64, :],
                                in1=bc[:], op=mybir.AluOpType.mult,
                            )
                        else:
                            om = rpool.tile([64, 512], BF16, name="om",
                                            tag="om")
                            nc.vector.tensor_tensor(
                                out=om[:], in0=usrc[0:64, :], in1=bc[:],
                                op=mybir.AluOpType.mult,
                            )
                            nc.sync.dma_start(ot[c][hp][64:128, :], om[:])
                    lp.__exit__(None, None, None)
                    if after_hp is not None:
                        after_hp(hp)

            # ---------------- schedule ----------------
            # chunk-0 Q/K projections run up front (attention needs them);
            # everything else flows through one continuous filler queue:
            #   during chunk 0: v0, qk1, v1
            #   during chunk 1: + qk2, v2
            #   during chunk 2: + qk3, op0
            #   during chunk 3: + v3, op1, op2 (+ op3 partials after hp2)
            # The adaptive pump quota leaves enough for the exp-bound late
            # chunks; the dependency-driven scheduler tolerates any slack.
            n_points = sum(4 * (2 * c + 2) for c in range(TC))
            fil = Filler(n_points)
            # hp0's q/k first, then v0 (hp0's AV ensures pull v0 through the
            # FIFO, so it must sit ahead of the later head-pairs' q/k)
            fil.add(proj_unit_gen(0, 0, 0), KD, key=("qk", 0, 0, 0))
            fil.add(proj_unit_gen(0, 1, 0), KD, key=("qk", 0, 1, 0))
            add_proj_v(fil, 0)
            for m in range(1, 4):
                fil.add(proj_unit_gen(0, 0, m), KD, key=("qk", 0, 0, m))
                fil.add(proj_unit_gen(0, 1, m), KD, key=("qk", 0, 1, m))
            add_proj_qk(fil, 1)
            add_proj_v(fil, 1)
            attn_chunk(0, fil)
            add_proj_qk(fil, 2)
            add_proj_v(fil, 2)
            attn_chunk(1, fil)
            add_proj_qk(fil, 3)
            add_outproj(fil, 0)
            attn_chunk(2, fil)
            add_proj_v(fil, 3)
            add_outproj(fil, 1)

            def after_hp3(hp):
                if hp == PR - 2:
                    for tt4 in range(4):
                        for n2 in range(2):
                            fil.add(outproj_partial_gen(TC - 1, tt4, n2),
                                    PR - 1)
                if hp == PR - 1:
                    # reserve: op2 emits right after the last AV group, so
                    # the PE grinds through it while the final normalization
                    # chain (cast/spread/recip/broadcast) runs on the other
                    # engines
                    add_outproj(fil, 2)

            attn_chunk(3, fil, after_hp=after_hp3)
            fil.drain()
            for tt4 in range(4):
                for n2 in range(2):
                    outproj_tail(TC - 1, tt4, n2)

    nc.compile()
    return nc


_NC_CACHE = None


def _get_nc():
    global _NC_CACHE
    if _NC_CACHE is None:
        _NC_CACHE = build_nc()
    return _NC_CACHE


def _shard_inputs(x, Wq, bq, Wk, bk, Wv, bv, Wp, bp):
    """Build the 8 per-core input maps."""
    bf16 = ml_dtypes.bfloat16
    x = np.asarray(x, dtype=np.float32)
    ca = np.ascontiguousarray
    in_maps = []
    for core in range(N_CORES):
        b, g = core // 2, core % 2
        cols = slice(g * DH, (g + 1) * DH)
        bq_g = np.asarray(bq[cols], np.float32).reshape(PR, P).T
        bk_g = np.asarray(bk[cols], np.float32).reshape(PR, P).T
        bv_g = np.broadcast_to(np.asarray(bv[cols], np.float32), (P, DH))
        if g == 0:
            bp_b = np.broadcast_to(np.asarray(bp, np.float32), (P, D))
        else:
            bp_b = np.zeros((P, D), np.float32)
        in_maps.append({
            "xt": ca(x[b].T.astype(bf16)),
            "wq": ca(np.asarray(Wq, np.float32)[:, cols].astype(bf16)),
            "wk": ca(np.asarray(Wk, np.float32)[:, cols].astype(bf16)),
            "wv": ca(np.asarray(Wv, np.float32)[:, cols].astype(bf16)),
            "bq": ca(bq_g),
            "bk": ca(bk_g),
            "bvb": ca(bv_g),
            "wp": ca(np.asarray(Wp, np.float32)[cols, :].astype(bf16)),
            "bpb": ca(bp_b),
        })
    return in_maps


def run_sharded(inputs, trace=False):
    """Run on 8 cores; returns (full_output, BassKernelResults)."""
    nc = _get_nc()
    in_maps = _shard_inputs(**inputs)
    res = run_bass_kernel_spmd(
        nc, in_maps, core_ids=list(range(N_CORES)), trace=trace
    )
    out = np.empty((B, T, D), np.float32)
    for b in range(B):
        out[b] = res.results[2 * b]["y"] + res.results[2 * b + 1]["y"]
    return out, res


def kernel(**inputs) -> np.ndarray:
    out, _ = run_sharded(inputs)
    return out


# revision 19
# speedup vs baseline: 2.3020x; 1.0024x over previous
"""Causal self-attention (B=4, T=2048, D=1024, H=16) on 8 trn2 NeuronCores.

Sharding: core c -> (batch b = c // 2, head-group g = c % 2). Each core runs
one batch element with 8 of the 16 heads: column-sharded Wq/Wk/Wv, row-sharded
Wp. Per-core output is a partial product of the output projection; the host
sums the two head-group partials per batch (bp is added on-device by group 0
via a broadcast input; group 1 gets zeros).

v3: on top of v2's software-pipelined schedule:
  - DMA head: x chunk 0 rides the gpsimd queue alone while the weights stream
    on the sync queue in need-order (wq, wk, wv, wp, bpb) -- the first
    projection matmul starts ~9us earlier.
  - Causal mask: a static 128x128 lower-triangle bf16 mask multiplied in by
    the DVE on just the boundary block of each diagonal tile (the i=1 member
    of a diagonal pair also shrinks its scores/exp/AV ranges by 128 cols, so
    no memset is needed). Replaces the per-tile gpsimd affine_select over the
    whole [*, 2, 512-r0] region: shorter exp->AV latency, gpsimd off the
    critical path, ~3us less PE work.
  - Softmax normalization: the denominator spread (r4) DMAs straight out of
    PSUM in parallel with the U copy; the final head-pair skips the copy
    entirely (DVE multiply reads PSUM) since no later AV needs the bank.
  - Filler: one continuous queue (no per-chunk drains) with an adaptive
    pump quota, and the output projections deferred (op0 fills chunk-2,
    op1+op2+v3 fill chunk-3) so the exp-bound last chunk keeps the PE fed.
  - Last chunk's output projection split pr0-2 / pr3: the pr0-2 partials and
    bias accumulate into SBUF while hp3 is still running; only 8 pr=3
    matmuls + adds + stores remain after the last normalization.
All matmul operands are stored bf16 (PSUM accumulation stays fp32).
"""

import numpy as np
import ml_dtypes

import concourse.mybir as mybir
import concourse.tile as tile
from concourse import bacc
from concourse.bass_utils import run_bass_kernel_spmd

B, T, D, H_FULL = 4, 2048, 1024, 16
H = H_FULL // 2          # heads per core
HD = 64                  # head dim
DH = H * HD              # 512, per-core head width
P = 128
TT = T // P              # 16 t tiles
TC = T // 512            # 4 t chunks
KD = D // P              # 8 contraction tiles over D
PR = H // 2              # 4 head pairs
N_CORES = 8

F32 = mybir.dt.float32
BF16 = mybir.dt.bfloat16


class Filler:
    """Queue of deferred PE work (projections / output projection), emitted
    in small bites between attention matmul groups so the in-order PE queue
    always has independent work while ACT runs exp."""

    def __init__(self, total_points):
        self.units = []      # list of (key, generator) pairs
        self.cur = None
        self.cur_key = None
        self.mms = 0         # matmuls remaining (approximate pacing weight)
        self.acc = 0.0
        self.done_keys = set()
        self.points_left = total_points

    def add(self, gen, n_mms, key=None):
        self.units.append((key, gen))
        self.mms += n_mms

    def pump(self, n):
        """Emit work until n matmuls have been issued (or queue empty)."""
        done = 0
        while done < n:
            if self.cur is None:
                if not self.units:
                    return
                self.cur_key, self.cur = self.units.pop(0)
            for kind, thunk in self.cur:
                thunk()
                if kind == "mm":
                    self.mms -= 1
                    done += 1
                    if done >= n:
                        break
            else:
                self.done_keys.add(self.cur_key)
                self.cur = None

    def ensure(self, key):
        """Emit whole units until the unit tagged `key` has been fully
        emitted. Emission order IS program order -- a consumer emitted
        before its producer reads stale data -- so anything an upcoming
        instruction reads must be forced out of the queue first."""
        if key in self.done_keys or not any(
                k == key for k, _ in self.units) and self.cur_key != key:
            return
        while key not in self.done_keys and (self.cur or self.units):
            if self.cur is None:
                self.cur_key, self.cur = self.units.pop(0)
            for kind, thunk in self.cur:
                thunk()
                if kind == "mm":
                    self.mms -= 1
            self.done_keys.add(self.cur_key)
            self.cur = None

    def pump_point(self):
        """One pacing point: emit enough matmuls to cover the exp-vs-PE
        deficit of one tp-pair, scaled up when the queue is deep relative
        to the remaining pacing points (late chunks)."""
        quota = max(4.0, self.mms / max(self.points_left, 1))
        quota = min(quota, 8.0)
        self.points_left -= 1
        self.acc += quota
        n = int(self.acc)
        if n > 0:
            self.acc -= n
            self.pump(n)

    def drain(self):
        self.pump(1 << 30)


def build_nc():
    nc = bacc.Bacc(None, target_bir_lowering=False)

    xt = nc.dram_tensor("xt", [D, T], BF16, kind="ExternalInput")
    wq = nc.dram_tensor("wq", [D, DH], BF16, kind="ExternalInput")
    wk = nc.dram_tensor("wk", [D, DH], BF16, kind="ExternalInput")
    wv = nc.dram_tensor("wv", [D, DH], BF16, kind="ExternalInput")
    bq = nc.dram_tensor("bq", [P, PR], F32, kind="ExternalInput")
    bk = nc.dram_tensor("bk", [P, PR], F32, kind="ExternalInput")
    bvb = nc.dram_tensor("bvb", [P, DH], F32, kind="ExternalInput")
    wp = nc.dram_tensor("wp", [DH, D], BF16, kind="ExternalInput")
    bpb = nc.dram_tensor("bpb", [P, D], F32, kind="ExternalInput")
    y = nc.dram_tensor("y", [T, D], F32, kind="ExternalOutput")

    xt_r = xt.rearrange("(o p) t -> p o t", p=P)
    wq_r = wq.rearrange("(o p) f -> p o f", p=P)
    wk_r = wk.rearrange("(o p) f -> p o f", p=P)
    wv_r = wv.rearrange("(o p) f -> p o f", p=P)

    with tile.TileContext(nc) as tc:
        with (
            tc.tile_pool(name="persist", bufs=1) as pp,
            tc.tile_pool(name="xpool", bufs=2) as xpool,
            tc.tile_pool(name="epool", bufs=6) as epool,
            tc.tile_pool(name="upool", bufs=4) as upool,
            tc.tile_pool(name="rpool", bufs=2) as rpool,
            tc.tile_pool(name="ypool", bufs=3) as ypool,
            tc.tile_pool(name="ypart", bufs=8) as ypartp,
            tc.tile_pool(name="work", bufs=2, space="PSUM") as work,
            tc.tile_pool(name="psS", bufs=2, space="PSUM") as psS,
            tc.tile_pool(name="psU0", bufs=1, space="PSUM") as psU0,
            tc.tile_pool(name="psU1", bufs=1, space="PSUM") as psU1,
        ):
            # x chunk 0 alone on the gpsimd DMA queue; weights queue on sync
            # in need-order so the first Q-proj matmul only waits for
            # x0 + wq (~2MB) instead of the whole 5.5MB input set.
            xt_tiles = {}

            def load_chunk_x(c):
                xt_tiles[c] = xpool.tile([P, KD, 512], BF16, name="xt_c",
                                         tag="xt")
                if c == 0:
                    # quartered so the first projection matmuls (dk 0..1)
                    # can start as soon as the first piece lands
                    for q4 in range(4):
                        nc.gpsimd.dma_start(
                            xt_tiles[0][:, 2 * q4:2 * q4 + 2, :],
                            xt_r[:, 2 * q4:2 * q4 + 2, 0:512])
                    return
                if c == 1:
                    # gate x1 behind phase 2 (reads wv_s, writes into the
                    # x1 region that the real load then overwrites)
                    nc.gpsimd.dma_start(
                        xt_tiles[1][0:1, 0, 0:3], wv_s[0:1, 0, 6:9])
                nc.gpsimd.dma_start(
                    xt_tiles[c][:], xt_r[:, :, c * 512:(c + 1) * 512])

            load_chunk_x(0)

            bq_s = pp.tile([P, PR], F32, name="bq_s")
            nc.sync.dma_start(bq_s[:], bq[:])
            bk_s = pp.tile([P, PR], F32, name="bk_s")
            nc.sync.dma_start(bk_s[:], bk[:])
            bvb_s = pp.tile([P, DH], F32, name="bvb_s")
            nc.sync.dma_start(bvb_s[:], bvb[:])

            wq_s = pp.tile([P, KD, DH], BF16, name="wq_s")
            wk_s = pp.tile([P, KD, DH], BF16, name="wk_s")
            wv_s = pp.tile([P, KD, DH], BF16, name="wv_s")
            wp_s = pp.tile([P, PR, D], BF16, name="wp_s")
            bpb_s = pp.tile([P, D], F32, name="bpb_s")
            # HBM bandwidth is shared round-robin across in-flight transfers,
            # so a flat issue order starves the first-needed data. The loads
            # are split per head-pair column slice (the q/k proj unit for
            # head-pair m reads only cols [m*128,(m+1)*128)) and staged in
            # phases. Phase boundaries are enforced with tiny "canary" DMAs
            # whose DESTINATION overlaps the next phase's tile (a WAW data
            # dependency the scheduler must honor) and whose source is the
            # previous phase's data (so the canary waits for it to land).
            def wslice(dst, srcr, m):
                nc.sync.dma_start(dst[:, :, m * P:(m + 1) * P],
                                  srcr[:, :, m * P:(m + 1) * P])

            # phase 1: x0 (issued above) + wq/wk head-pair 0
            wslice(wq_s, wq_r, 0)
            wslice(wk_s, wk_r, 0)
            # phase 2 canaries: dst overlaps the start of each gated slice
            # (the real load then overwrites the canary bytes), src reads
            # phase-1 data so the canary waits for it to land
            nc.sync.dma_start(wq_s[0:1, 0, P:DH:P],
                              xt_tiles[0][0:1, 0, 0:3])
            nc.sync.dma_start(wk_s[0:1, 0, P:DH:P],
                              xt_tiles[0][0:1, 0, 4:7])
            nc.sync.dma_start(wv_s[0:1, 0, 0:3], xt_tiles[0][0:1, 0, 8:11])
            nc.sync.dma_start(wv_s[0:1, 4, 0:3], xt_tiles[0][0:1, 0, 12:15])
            # wv first: hp0's AV matmuls need V0 before hp1 needs wq/wk m1
            nc.sync.dma_start(wv_s[:, 0:4, :], wv_r[:, 0:4, :])
            nc.sync.dma_start(wv_s[:, 4:8, :], wv_r[:, 4:8, :])
            for m in range(1, 4):
                wslice(wq_s, wq_r, m)
                wslice(wk_s, wk_r, m)
            # phase 3 canaries: gated on wv
            nc.sync.dma_start(wp_s[0:1, 0, 0:3], wv_s[0:1, 0, 3:6])
            nc.sync.dma_start(
                bpb_s.bitcast(mybir.dt.uint8)[0:1, 0:12],
                wv_s.bitcast(mybir.dt.uint8)[0:1, 0:1, 12:24])
            nc.sync.dma_start(wp_s[:], wp.rearrange("(o p) f -> p o f", p=P))
            nc.sync.dma_start(bpb_s[:], bpb[:])

            # per-chunk tensors (separate tiles -> exact dependency tracking
            # so interleaved chunks never falsely serialize); ot additionally
            # per head-pair so the output projection's pr-accumulation chain
            # can start as soon as the first pair is normalized
            qt = [pp.tile([P, PR, 512], BF16, name=f"qt{c}") for c in range(TC)]
            kt = [pp.tile([P, PR, 512], BF16, name=f"kt{c}") for c in range(TC)]
            vv = [pp.tile([P, 4, H, HD + 1], BF16, name=f"vv{c}")
                  for c in range(TC)]
            ot = [[pp.tile([P, 512], BF16, name=f"ot{c}_{pr}")
                   for pr in range(PR)] for c in range(TC)]
            for c in range(TC):
                nc.any.memset(vv[c][:, :, :, HD], 1.0)

            # static lower-triangle mask for the diagonal boundary blocks:
            # tri[p, j, col] = 1 if col >= p else 0 (same for both heads j)
            tri = pp.tile([P, 2, P], BF16, name="tri")
            nc.any.memset(tri[:], 1.0)
            nc.gpsimd.affine_select(
                out=tri[:], in_=tri[:],
                compare_op=mybir.AluOpType.is_ge,
                fill=0.0, base=0, pattern=[[0, 2], [1, P]],
                channel_multiplier=-1,
            )

            def proj_unit_gen(c, kind, m):
                """One projection subunit: 8 accumulating matmuls + bias add.
                kind: 0=Q, 1=K, 2=V(m = t4)."""
                xt_c = xt_tiles[c]
                pq = work.tile([P, 512], F32, name="pq", tag="pp")
                if kind < 2:
                    w_s = (wq_s, wk_s)[kind]
                    for dk in range(KD):
                        yield ("mm", (lambda dk=dk: nc.tensor.matmul(
                            pq[:],
                            w_s[:, dk, m * P:(m + 1) * P],
                            xt_c[:, dk, :],
                            start=(dk == 0),
                            stop=(dk == KD - 1),
                        )))
                    dst = (qt, kt)[kind]
                    b_s = (bq_s, bk_s)[kind]
                    if c < 3:
                        # per-partition-scalar bias: legal on the scalar
                        # engine, which has slack here -- keeps the DVE
                        # queue short so PSUM-pool reuse never gates the PE
                        yield ("free", (lambda: nc.scalar.add(
                            out=dst[c][:, m, :],
                            in_=pq[:],
                            add=b_s[:, m, None],
                        )))
                    else:
                        yield ("free", (lambda: nc.vector.tensor_tensor(
                            out=dst[c][:, m, :],
                            in0=pq[:],
                            in1=b_s[:, m, None].to_broadcast((P, 512)),
                            op=mybir.AluOpType.add,
                        )))
                else:
                    for dk in range(KD):
                        yield ("mm", (lambda dk=dk: nc.tensor.matmul(
                            pq[:],
                            xt_c[:, dk, m * P:(m + 1) * P],
                            wv_s[:, dk, :],
                            start=(dk == 0),
                            stop=(dk == KD - 1),
                        )))
                    yield ("free", (lambda: nc.vector.tensor_tensor(
                        out=vv[c][:, m, :, 0:HD],
                        in0=pq.rearrange("p (h d) -> p h d", h=H),
                        in1=bvb_s.rearrange("p (h d) -> p h d", h=H),
                        op=mybir.AluOpType.add,
                    )))

            def outproj_unit_gen(c, tt4, n2):
                """One output-projection subunit: 4 accumulating matmuls +
                bias add + store. tt4 = t-tile within chunk, n2 = D half."""
                tt_ = 4 * c + tt4
                ts_ = slice(tt_ * P, (tt_ + 1) * P)
                ns = slice(n2 * 512, (n2 + 1) * 512)
                py = work.tile([P, 512], F32, name="py", tag="pp")
                for pr in range(PR):
                    yield ("mm", (lambda pr=pr: nc.tensor.matmul(
                        py[:],
                        ot[c][pr][:, tt4 * P:(tt4 + 1) * P],
                        wp_s[:, pr, ns],
                        start=(pr == 0),
                        stop=(pr == PR - 1),
                    )))
                yt = ypool.tile([P, 512], F32, name="yt", tag="yt")
                yield ("free", (lambda: nc.vector.tensor_tensor(
                    out=yt[:], in0=py[:], in1=bpb_s[:, ns],
                    op=mybir.AluOpType.add,
                )))
                yield ("free", (lambda: nc.gpsimd.dma_start(y[ts_, ns],
                                                            yt[:])))

            # last-chunk output projection, split so only the pr=3 matmul
            # trails the final normalization
            ypart_tiles = {}

            def outproj_partial_gen(c, tt4, n2):
                """pr 0..2 accumulation + bias -> fp32 SBUF partial."""
                ns = slice(n2 * 512, (n2 + 1) * 512)
                py = work.tile([P, 512], F32, name="py", tag="pp")
                for pr in range(PR - 1):
                    yield ("mm", (lambda pr=pr: nc.tensor.matmul(
                        py[:],
                        ot[c][pr][:, tt4 * P:(tt4 + 1) * P],
                        wp_s[:, pr, ns],
                        start=(pr == 0),
                        stop=(pr == PR - 2),
                    )))
                yp = ypartp.tile([P, 512], F32, name="yp", tag="yp")
                ypart_tiles[(tt4, n2)] = yp
                yield ("free", (lambda: nc.vector.tensor_tensor(
                    out=yp[:], in0=py[:], in1=bpb_s[:, ns],
                    op=mybir.AluOpType.add,
                )))

            def outproj_tail(c, tt4, n2):
                tt_ = 4 * c + tt4
                ts_ = slice(tt_ * P, (tt_ + 1) * P)
                ns = slice(n2 * 512, (n2 + 1) * 512)
                py = work.tile([P, 512], F32, name="py2", tag="pp")
                nc.tensor.matmul(
                    py[:],
                    ot[c][PR - 1][:, tt4 * P:(tt4 + 1) * P],
                    wp_s[:, PR - 1, ns],
                    start=True, stop=True,
                )
                yt = ypool.tile([P, 512], F32, name="yt", tag="yt")
                nc.vector.tensor_tensor(
                    out=yt[:], in0=py[:], in1=ypart_tiles[(tt4, n2)][:],
                    op=mybir.AluOpType.add,
                )
                nc.sync.dma_start(y[ts_, ns], yt[:])

            def add_proj_qk(fil, c):
                if c not in xt_tiles:
                    load_chunk_x(c)
                for m in range(4):
                    for kind in range(2):
                        fil.add(proj_unit_gen(c, kind, m), KD,
                                key=("qk", c, kind, m))

            def add_proj_v(fil, c):
                for m in range(4):
                    fil.add(proj_unit_gen(c, 2, m), KD, key=("v", c, m))

            def add_outproj(fil, c):
                for tt4 in range(4):
                    for n2 in range(2):
                        fil.add(outproj_unit_gen(c, tt4, n2), PR)

            def attn_chunk(c, fil, after_hp=None):
                ntk = 4 * c + 4
                for hp in range(PR):
                    # program order is emission order: this head-pair's q/k
                    # projection units (hp-major FIFO order, so ensuring the
                    # k unit flushes the q unit too) must be emitted before
                    # its first scores matmul
                    fil.ensure(("qk", c, 1, hp))
                    ups = [
                        (psU0 if j == 0 else psU1).tile(
                            [HD + 1, 512], F32, name=f"up{j}", tag=f"u{j}")
                        for j in (0, 1)
                    ]
                    for tp in range(0, ntk, 2):
                        diag = tp >= 4 * c
                        r0 = P * (tp - 4 * c) if diag else 0
                        sps, ets = [], []
                        for i in (0, 1):
                            sps.append(psS.tile(
                                [P, 2, 512], F32, name="sp", tag="s"))
                            ets.append(epool.tile(
                                [P, 2, 512], BF16, name="et", tag="e"))
                        for i in (0, 1):
                            t = tp + i
                            tc_, t4 = t // 4, t % 4
                            ri = r0 + P * i if diag else 0
                            for j in (0, 1):
                                # j=0 rows 0-63, j=1 rows 64-127: disjoint
                                # row groups run concurrently on the PE
                                pb = 64 * j
                                nc.tensor.matmul(
                                    sps[i][:, j, ri:512],
                                    kt[tc_][pb:pb + 64, hp,
                                            t4 * P:(t4 + 1) * P],
                                    qt[c][pb:pb + 64, hp, ri:512],
                                    start=True,
                                    stop=True,
                                )
                        for i in (0, 1):
                            ri = r0 + P * i if diag else 0
                            nc.scalar.activation(
                                ets[i][:, :, ri:512], sps[i][:, :, ri:512],
                                mybir.ActivationFunctionType.Exp,
                                scale=float(1.0 / np.sqrt(HD)),
                            )
                            if diag:
                                # zero the upper triangle of the boundary
                                # 128-block with a static-mask DVE multiply
                                # (same mask for both heads)
                                nc.vector.tensor_tensor(
                                    out=ets[i][:, :, ri:ri + P],
                                    in0=ets[i][:, :, ri:ri + P],
                                    in1=tri[:],
                                    op=mybir.AluOpType.mult,
                                )
                        # filler between scores/exp and the dependent AV
                        # matmuls: the PE would otherwise stall here
                        fil.pump_point()
                        for i in (0, 1):
                            # producers of vv must be emitted before the AV
                            # matmuls that read them (emission order is
                            # program order)
                            t = tp + i
                            fil.ensure(("v", t // 4, t % 4))
                        for i in (0, 1):
                            t = tp + i
                            tc_, t4 = t // 4, t % 4
                            ri = r0 + P * i if diag else 0
                            for j in (0, 1):
                                nc.tensor.matmul(
                                    ups[j][:, ri:512],
                                    vv[tc_][:, t4, 2 * hp + j, :],
                                    ets[i][:, j, ri:512],
                                    start=(t == 0),
                                    stop=(t == ntk - 1),
                                )
                    # softmax normalization. The reciprocal of the denominator
                    # row is computed via the DMA-spread trick; the spread
                    # reads straight from PSUM so it runs concurrently with
                    # the U copy. The last head-pair of the last chunk skips
                    # the copy (multiply reads PSUM) -- nothing needs the
                    # banks afterwards.
                    lp = nc.allow_low_precision(
                        reason="bf16 softmax normalization; rel tol 2e-2")
                    lp.__enter__()
                    for j in (0, 1):
                        uu = upool.tile([HD + 1, 512], BF16, name="uu",
                                        tag=f"uu{j}")
                        # denominator row first: the spread DMA only waits on
                        # this small copy, not the full-U cast
                        nc.vector.tensor_copy(uu[HD:HD + 1, :],
                                              ups[j][HD:HD + 1, :])
                        r4 = rpool.tile([32, 16], BF16, name="r4", tag="r4")
                        nc.sync.dma_start(r4[:], uu[HD:HD + 1, :])
                        nc.vector.tensor_copy(uu[0:HD, :], ups[j][0:HD, :])
                        usrc = uu
                        r4r = rpool.tile([32, 16], BF16, name="r4r",
                                         tag="r4r")
                        nc.vector.reciprocal(r4r[:], r4[:])
                        rb = rpool.tile([1, 512], BF16, name="rb", tag="rb")
                        nc.sync.dma_start(rb[:], r4r[:])
                        bc = rpool.tile([64, 512], BF16, name="bc",
                                        tag=f"bc{j}")
                        nc.gpsimd.partition_broadcast(bc[:], rb[0:1, :])
                        if j == 0:
                            nc.vector.tensor_tensor(
                                out=ot[c][hp][0:64, :], in0=usrc[0:64, :],
                                in1=bc[:], op=mybir.AluOpType.mult,
                            )
                        else:
                            om = rpool.tile([64, 512], BF16, name="om",
                                            tag="om")
                            nc.vector.tensor_tensor(
                                out=om[:], in0=usrc[0:64, :], in1=bc[:],
                                op=mybir.AluOpType.mult,
                            )
                            nc.sync.dma_start(ot[c][hp][64:128, :], om[:])
                    lp.__exit__(None, None, None)
                    if after_hp is not None:
                        after_hp(hp)

            # ---------------- schedule ----------------
            # chunk-0 Q/K projections run up front (attention needs them);
            # everything else flows through one continuous filler queue:
            #   during chunk 0: v0, qk1, v1
            #   during chunk 1: + qk2, v2
            #   during chunk 2: + qk3, op0
            #   during chunk 3: + v3, op1, op2 (+ op3 partials after hp2)
            # The adaptive pump quota leaves enough for the exp-bound late
            # chunks; the dependency-driven scheduler tolerates any slack.
            n_points = sum(4 * (2 * c + 2) for c in range(TC))
            fil = Filler(n_points)
            # hp0's q/k first, then v0 (hp0's AV ensures pull v0 through the
            # FIFO, so it must sit ahead of the later head-pairs' q/k)
            fil.add(proj_unit_gen(0, 0, 0), KD, key=("qk", 0, 0, 0))
            fil.add(proj_unit_gen(0, 1, 0), KD, key=("qk", 0, 1, 0))
            add_proj_v(fil, 0)
            for m in range(1, 4):
                fil.add(proj_unit_gen(0, 0, m), KD, key=("qk", 0, 0, m))
                fil.add(proj_unit_gen(0, 1, m), KD, key=("qk", 0, 1, m))
            add_proj_qk(fil, 1)
            add_proj_v(fil, 1)
            attn_chunk(0, fil)
            add_proj_qk(fil, 2)
            add_proj_v(fil, 2)
            attn_chunk(1, fil)
            add_proj_qk(fil, 3)
            add_outproj(fil, 0)
            attn_chunk(2, fil)
            add_proj_v(fil, 3)
            add_outproj(fil, 1)

            def after_hp3(hp):
                if hp == PR - 2:
                    for tt4 in range(4):
                        for n2 in range(2):
                            fil.add(outproj_partial_gen(TC - 1, tt4, n2),
                                    PR - 1)
                if hp == PR - 1:
                    # reserve: op2 emits right after the last AV group, so
                    # the PE grinds through it while the final normalization
                    # chain (cast/spread/recip/broadcast) runs on the other
                    # engines
                    add_outproj(fil, 2)

            attn_chunk(3, fil, after_hp=after_hp3)
            fil.drain()
            for tt4 in range(4):
                for n2 in range(2):
                    outproj_tail(TC - 1, tt4, n2)

    nc.compile()
    return nc


_NC_CACHE = None


def _get_nc():
    global _NC_CACHE
    if _NC_CACHE is None:
        _NC_CACHE = build_nc()
    return _NC_CACHE


def _shard_inputs(x, Wq, bq, Wk, bk, Wv, bv, Wp, bp):
    """Build the 8 per-core input maps."""
    bf16 = ml_dtypes.bfloat16
    x = np.asarray(x, dtype=np.float32)
    ca = np.ascontiguousarray
    in_maps = []
    for core in range(N_CORES):
        b, g = core // 2, core % 2
        cols = slice(g * DH, (g + 1) * DH)
        bq_g = np.asarray(bq[cols], np.float32).reshape(PR, P).T
        bk_g = np.asarray(bk[cols], np.float32).reshape(PR, P).T
        bv_g = np.broadcast_to(np.asarray(bv[cols], np.float32), (P, DH))
        if g == 0:
            bp_b = np.broadcast_to(np.asarray(bp, np.float32), (P, D))
        else:
            bp_b = np.zeros((P, D), np.float32)
        in_maps.append({
            "xt": ca(x[b].T.astype(bf16)),
            "wq": ca(np.asarray(Wq, np.float32)[:, cols].astype(bf16)),
            "wk": ca(np.asarray(Wk, np.float32)[:, cols].astype(bf16)),
            "wv": ca(np.asarray(Wv, np.float32)[:, cols].astype(bf16)),
            "bq": ca(bq_g),
            "bk": ca(bk_g),
            "bvb": ca(bv_g),
            "wp": ca(np.asarray(Wp, np.float32)[cols, :].astype(bf16)),
            "bpb": ca(bp_b),
        })
    return in_maps


def run_sharded(inputs, trace=False):
    """Run on 8 cores; returns (full_output, BassKernelResults)."""
    nc = _get_nc()
    in_maps = _shard_inputs(**inputs)
    res = run_bass_kernel_spmd(
        nc, in_maps, core_ids=list(range(N_CORES)), trace=trace
    )
    out = np.empty((B, T, D), np.float32)
    for b in range(B):
        out[b] = res.results[2 * b]["y"] + res.results[2 * b + 1]["y"]
    return out, res


def kernel(**inputs) -> np.ndarray:
    out, _ = run_sharded(inputs)
    return out


# revision 28
# speedup vs baseline: 2.3406x; 1.0168x over previous
"""Causal self-attention (B=4, T=2048, D=1024, H=16) on 8 trn2 NeuronCores.

Sharding: core c -> (batch b = c // 2, head-group g = c % 2). Each core runs
one batch element with 8 of the 16 heads: column-sharded Wq/Wk/Wv, row-sharded
Wp. Per-core output is a partial product of the output projection; the host
sums the two head-group partials per batch (bp is added on-device by group 0
via a broadcast input; group 1 gets zeros).

v3: on top of v2's software-pipelined schedule:
  - DMA head: x chunk 0 rides the gpsimd queue alone while the weights stream
    on the sync queue in need-order (wq, wk, wv, wp, bpb) -- the first
    projection matmul starts ~9us earlier.
  - Causal mask: a static 128x128 lower-triangle bf16 mask multiplied in by
    the DVE on just the boundary block of each diagonal tile (the i=1 member
    of a diagonal pair also shrinks its scores/exp/AV ranges by 128 cols, so
    no memset is needed). Replaces the per-tile gpsimd affine_select over the
    whole [*, 2, 512-r0] region: shorter exp->AV latency, gpsimd off the
    critical path, ~3us less PE work.
  - Softmax normalization: the denominator spread (r4) DMAs straight out of
    PSUM in parallel with the U copy; the final head-pair skips the copy
    entirely (DVE multiply reads PSUM) since no later AV needs the bank.
  - Filler: one continuous queue (no per-chunk drains) with an adaptive
    pump quota, and the output projections deferred (op0 fills chunk-2,
    op1+op2+v3 fill chunk-3) so the exp-bound last chunk keeps the PE fed.
  - Last chunk's output projection split pr0-2 / pr3: the pr0-2 partials and
    bias accumulate into SBUF while hp3 is still running; only 8 pr=3
    matmuls + adds + stores remain after the last normalization.
All matmul operands are stored bf16 (PSUM accumulation stays fp32).
"""

import numpy as np
import ml_dtypes

import concourse.mybir as mybir
import concourse.tile as tile
from concourse import bacc
from concourse.bass_utils import run_bass_kernel_spmd

B, T, D, H_FULL = 4, 2048, 1024, 16
H = H_FULL // 2          # heads per core
HD = 64                  # head dim
DH = H * HD              # 512, per-core head width
P = 128
TT = T // P              # 16 t tiles
TC = T // 512            # 4 t chunks
KD = D // P              # 8 contraction tiles over D
PR = H // 2              # 4 head pairs
N_CORES = 8

F32 = mybir.dt.float32
BF16 = mybir.dt.bfloat16


class Filler:
    """Queue of deferred PE work (projections / output projection), emitted
    in small bites between attention matmul groups so the in-order PE queue
    always has independent work while ACT runs exp."""

    def __init__(self, total_points):
        self.units = []      # list of (key, generator) pairs
        self.cur = None
        self.cur_key = None
        self.mms = 0         # matmuls remaining (approximate pacing weight)
        self.acc = 0.0
        self.done_keys = set()
        self.points_left = total_points

    def add(self, gen, n_mms, key=None):
        self.units.append((key, gen))
        self.mms += n_mms

    def pump(self, n):
        """Emit work until n matmuls have been issued (or queue empty)."""
        done = 0
        while done < n:
            if self.cur is None:
                if not self.units:
                    return
                self.cur_key, self.cur = self.units.pop(0)
            for kind, thunk in self.cur:
                thunk()
                if kind == "mm":
                    self.mms -= 1
                    done += 1
                    if done >= n:
                        break
            else:
                self.done_keys.add(self.cur_key)
                self.cur = None

    def ensure(self, key):
        """Emit whole units until the unit tagged `key` has been fully
        emitted. Emission order IS program order -- a consumer emitted
        before its producer reads stale data -- so anything an upcoming
        instruction reads must be forced out of the queue first."""
        if key in self.done_keys or not any(
                k == key for k, _ in self.units) and self.cur_key != key:
            return
        while key not in self.done_keys and (self.cur or self.units):
            if self.cur is None:
                self.cur_key, self.cur = self.units.pop(0)
            for kind, thunk in self.cur:
                thunk()
                if kind == "mm":
                    self.mms -= 1
            self.done_keys.add(self.cur_key)
            self.cur = None

    def pump_point(self):
        """One pacing point: emit enough matmuls to cover the exp-vs-PE
        deficit of one tp-pair, scaled up when the queue is deep relative
        to the remaining pacing points (late chunks)."""
        quota = max(4.0, self.mms / max(self.points_left, 1))
        quota = min(quota, 8.0)
        self.points_left -= 1
        self.acc += quota
        n = int(self.acc)
        if n > 0:
            self.acc -= n
            self.pump(n)

    def drain(self):
        self.pump(1 << 30)


def build_nc():
    nc = bacc.Bacc(None, target_bir_lowering=False)

    # all large inputs are host-permuted into SBUF layout so every DMA is
    # contiguous 2-8KB per partition line (the on-device rearranges cost ~2x
    # in DMA efficiency: 256B-1KB lines)
    xt = nc.dram_tensor("xt", [P, TC, KD, 512], BF16, kind="ExternalInput")
    wq = nc.dram_tensor("wq", [P, PR, KD, P], BF16, kind="ExternalInput")
    wk = nc.dram_tensor("wk", [P, PR, KD, P], BF16, kind="ExternalInput")
    wv = nc.dram_tensor("wv", [P, KD, DH], BF16, kind="ExternalInput")
    bq = nc.dram_tensor("bq", [P, PR], F32, kind="ExternalInput")
    bk = nc.dram_tensor("bk", [P, PR], F32, kind="ExternalInput")
    bvb = nc.dram_tensor("bvb", [P, DH], F32, kind="ExternalInput")
    wp = nc.dram_tensor("wp", [P, PR, D], BF16, kind="ExternalInput")
    bpb = nc.dram_tensor("bpb", [P, D], F32, kind="ExternalInput")
    y = nc.dram_tensor("y", [T, D], F32, kind="ExternalOutput")


    with tile.TileContext(nc) as tc:
        with (
            tc.tile_pool(name="persist", bufs=1) as pp,
            tc.tile_pool(name="xpool", bufs=2) as xpool,
            tc.tile_pool(name="epool", bufs=8) as epool,
            tc.tile_pool(name="upool", bufs=6) as upool,
            tc.tile_pool(name="rpool", bufs=3) as rpool,
            tc.tile_pool(name="ypool", bufs=4) as ypool,
            tc.tile_pool(name="ypart", bufs=8) as ypartp,
            tc.tile_pool(name="work", bufs=2, space="PSUM") as work,
            tc.tile_pool(name="psS", bufs=2, space="PSUM") as psS,
            tc.tile_pool(name="psU0", bufs=1, space="PSUM") as psU0,
            tc.tile_pool(name="psU1", bufs=1, space="PSUM") as psU1,
        ):
            # x chunk 0 alone on the gpsimd DMA queue; weights queue on sync
            # in need-order so the first Q-proj matmul only waits for
            # x0 + wq (~2MB) instead of the whole 5.5MB input set.
            xt_tiles = {}
            x0_pieces = [pp.tile([P, 2, 512], BF16, name=f"x0p{q4}")
                         for q4 in range(4)]

            def xt_ap(c, dk):
                if c == 0:
                    return x0_pieces[dk // 2][:, dk % 2, :]
                return xt_tiles[c][:, dk, :]

            def load_chunk_x(c):
                if c == 0:
                    # separate piece tiles: the first projection matmuls
                    # (dk 0..1) start as soon as the first piece lands
                    for q4 in range(4):
                        nc.gpsimd.dma_start(
                            x0_pieces[q4][:], xt[:, 0, 2 * q4:2 * q4 + 2, :])
                    return
                xt_tiles[c] = xpool.tile([P, KD, 512], BF16, name="xt_c",
                                         tag="xt")
                if c == 1:
                    # gate x1 behind phase 2 (reads wv, writes into the
                    # x1 region that the real load then overwrites)
                    nc.gpsimd.dma_start(
                        xt_tiles[1][0:1, 0, 0:3], wv_tiles[1][0:1, 0, 0:3])
                nc.gpsimd.dma_start(xt_tiles[c][:], xt[:, c])

            load_chunk_x(0)

            bq_s = pp.tile([P, PR], F32, name="bq_s")
            nc.sync.dma_start(bq_s[:], bq[:])
            bk_s = pp.tile([P, PR], F32, name="bk_s")
            nc.sync.dma_start(bk_s[:], bk[:])
            bvb_s = pp.tile([P, DH], F32, name="bvb_s")
            nc.sync.dma_start(bvb_s[:], bvb[:])

            # per-slice weight tiles: exact dependencies (a consumer waits
            # only for its own slice) and contiguous host-permuted DMAs.
            # HBM bandwidth is shared round-robin across in-flight transfers,
            # so a flat issue order starves the first-needed data: loads are
            # staged in phases, enforced by tiny "canary" DMAs whose
            # DESTINATION overlaps the gated tile (a WAW data dependency the
            # scheduler must honor) and whose source is previous-phase data.
            wq_tiles = [pp.tile([P, KD, P], BF16, name=f"wq{m}")
                        for m in range(PR)]
            wk_tiles = [pp.tile([P, KD, P], BF16, name=f"wk{m}")
                        for m in range(PR)]
            wv_tiles = [pp.tile([P, 4, DH], BF16, name=f"wv{h}")
                        for h in range(2)]
            wp_s = pp.tile([P, PR, D], BF16, name="wp_s")
            bpb_s = pp.tile([P, D], F32, name="bpb_s")
            # phase 1: x0 (issued above) + wq/wk head-pair 0
            nc.sync.dma_start(wq_tiles[0][:], wq[:, 0])
            nc.sync.dma_start(wk_tiles[0][:], wk[:, 0])
            # phase 2 (canary-gated on phase 1): wv rides the gpsimd queue
            # (short issue train: just the x loads) so it starts the moment
            # phase 1 lands -- the sync queue's ~25-instruction issue train
            # would delay it by several us. hp0's AV matmuls need V0 before
            # hp1 needs wq/wk m1.
            for h in range(2):
                nc.gpsimd.dma_start(wv_tiles[h][0:1, 0, 0:3],
                                    x0_pieces[3][0:1, 0, 4 * h:4 * h + 3])
                nc.gpsimd.dma_start(wv_tiles[h][:], wv[:, 4 * h:4 * h + 4, :])
            for m in range(1, 4):
                nc.sync.dma_start(wq_tiles[m][0:1, 0, 0:3],
                                  wq_tiles[0][0:1, 0, 0:3])
                nc.sync.dma_start(wq_tiles[m][:], wq[:, m])
                nc.sync.dma_start(wk_tiles[m][0:1, 0, 0:3],
                                  wk_tiles[0][0:1, 0, 0:3])
                nc.sync.dma_start(wk_tiles[m][:], wk[:, m])
            # phase 3 canaries: gated on wv
            nc.sync.dma_start(wp_s[0:1, 0, 0:3], wv_tiles[0][0:1, 0, 3:6])
            nc.sync.dma_start(
                bpb_s.bitcast(mybir.dt.uint8)[0:1, 0:12],
                wv_tiles[0].bitcast(mybir.dt.uint8)[0:1, 0:1, 12:24])
            nc.sync.dma_start(wp_s[:], wp[:])
            nc.sync.dma_start(bpb_s[:], bpb[:])

            # per-chunk tensors (separate tiles -> exact dependency tracking
            # so interleaved chunks never falsely serialize); ot additionally
            # per head-pair so the output projection's pr-accumulation chain
            # can start as soon as the first pair is normalized
            qt = [pp.tile([P, PR, 512], BF16, name=f"qt{c}") for c in range(TC)]
            kt = [pp.tile([P, PR, 512], BF16, name=f"kt{c}") for c in range(TC)]
            vv = [pp.tile([P, 4, H, HD + 1], BF16, name=f"vv{c}")
                  for c in range(TC)]
            ot = [[pp.tile([P, 512], BF16, name=f"ot{c}_{pr}")
                   for pr in range(PR)] for c in range(TC)]
            for c in range(TC):
                nc.any.memset(vv[c][:, :, :, HD], 1.0)

            # static lower-triangle mask for the diagonal boundary blocks:
            # tri[p, j, col] = 1 if col >= p else 0 (same for both heads j)
            tri = pp.tile([P, 2, P], BF16, name="tri")
            nc.any.memset(tri[:], 1.0)
            nc.gpsimd.affine_select(
                out=tri[:], in_=tri[:],
                compare_op=mybir.AluOpType.is_ge,
                fill=0.0, base=0, pattern=[[0, 2], [1, P]],
                channel_multiplier=-1,
            )

            def proj_unit_gen(c, kind, m):
                """One projection subunit: 8 accumulating matmuls + bias add.
                kind: 0=Q, 1=K, 2=V(m = t4)."""
                pq = work.tile([P, 512], F32, name="pq", tag="pp")
                if kind < 2:
                    w_m = (wq_tiles, wk_tiles)[kind][m]
                    for dk in range(KD):
                        yield ("mm", (lambda dk=dk: nc.tensor.matmul(
                            pq[:],
                            w_m[:, dk, :],
                            xt_ap(c, dk),
                            start=(dk == 0),
                            stop=(dk == KD - 1),
                        )))
                    dst = (qt, kt)[kind]
                    b_s = (bq_s, bk_s)[kind]
                    if c < 3:
                        # per-partition-scalar bias: legal on the scalar
                        # engine, which has slack here -- keeps the DVE
                        # queue short so PSUM-pool reuse never gates the PE
                        yield ("free", (lambda: nc.scalar.add(
                            out=dst[c][:, m, :],
                            in_=pq[:],
                            add=b_s[:, m, None],
                        )))
                    else:
                        yield ("free", (lambda: nc.vector.tensor_tensor(
                            out=dst[c][:, m, :],
                            in0=pq[:],
                            in1=b_s[:, m, None].to_broadcast((P, 512)),
                            op=mybir.AluOpType.add,
                        )))
                else:
                    for dk in range(KD):
                        yield ("mm", (lambda dk=dk: nc.tensor.matmul(
                            pq[:],
                            xt_ap(c, dk)[:, m * P:(m + 1) * P],
                            wv_tiles[dk // 4][:, dk % 4, :],
                            start=(dk == 0),
                            stop=(dk == KD - 1),
                        )))
                    yield ("free", (lambda: nc.vector.tensor_tensor(
                        out=vv[c][:, m, :, 0:HD],
                        in0=pq.rearrange("p (h d) -> p h d", h=H),
                        in1=bvb_s.rearrange("p (h d) -> p h d", h=H),
                        op=mybir.AluOpType.add,
                    )))

            def outproj_unit_gen(c, tt4, n2):
                """One output-projection subunit: 4 accumulating matmuls +
                bias add + store. tt4 = t-tile within chunk, n2 = D half."""
                tt_ = 4 * c + tt4
                ts_ = slice(tt_ * P, (tt_ + 1) * P)
                ns = slice(n2 * 512, (n2 + 1) * 512)
                py = work.tile([P, 512], F32, name="py", tag="pp")
                for pr in range(PR):
                    yield ("mm", (lambda pr=pr: nc.tensor.matmul(
                        py[:],
                        ot[c][pr][:, tt4 * P:(tt4 + 1) * P],
                        wp_s[:, pr, ns],
                        start=(pr == 0),
                        stop=(pr == PR - 1),
                    )))
                yt = ypool.tile([P, 512], F32, name="yt", tag="yt")
                yield ("free", (lambda: nc.vector.tensor_tensor(
                    out=yt[:], in0=py[:], in1=bpb_s[:, ns],
                    op=mybir.AluOpType.add,
                )))
                yield ("free", (lambda: nc.gpsimd.dma_start(y[ts_, ns],
                                                            yt[:])))

            # last-chunk output projection, split so only the pr=3 matmul
            # trails the final normalization
            ypart_tiles = {}

            def outproj_partial_gen(c, tt4, n2):
                """pr 0..2 accumulation + bias -> fp32 SBUF partial."""
                ns = slice(n2 * 512, (n2 + 1) * 512)
                py = work.tile([P, 512], F32, name="py", tag="pp")
                for pr in range(PR - 1):
                    yield ("mm", (lambda pr=pr: nc.tensor.matmul(
                        py[:],
                        ot[c][pr][:, tt4 * P:(tt4 + 1) * P],
                        wp_s[:, pr, ns],
                        start=(pr == 0),
                        stop=(pr == PR - 2),
                    )))
                yp = ypartp.tile([P, 512], F32, name="yp", tag="yp")
                ypart_tiles[(tt4, n2)] = yp
                yield ("free", (lambda: nc.vector.tensor_tensor(
                    out=yp[:], in0=py[:], in1=bpb_s[:, ns],
                    op=mybir.AluOpType.add,
                )))

            def outproj_tail(c, tt4, n2):
                tt_ = 4 * c + tt4
                ts_ = slice(tt_ * P, (tt_ + 1) * P)
                ns = slice(n2 * 512, (n2 + 1) * 512)
                py = work.tile([P, 512], F32, name="py2", tag="pp")
                nc.tensor.matmul(
                    py[:],
                    ot[c][PR - 1][:, tt4 * P:(tt4 + 1) * P],
                    wp_s[:, PR - 1, ns],
                    start=True, stop=True,
                )
                yt = ypool.tile([P, 512], F32, name="yt", tag="yt")
                nc.vector.tensor_tensor(
                    out=yt[:], in0=py[:], in1=ypart_tiles[(tt4, n2)][:],
                    op=mybir.AluOpType.add,
                )
                nc.sync.dma_start(y[ts_, ns], yt[:])

            def add_proj_qk(fil, c):
                if c not in xt_tiles:
                    load_chunk_x(c)
                for m in range(4):
                    for kind in range(2):
                        fil.add(proj_unit_gen(c, kind, m), KD,
                                key=("qk", c, kind, m))

            def add_proj_v(fil, c):
                for m in range(4):
                    fil.add(proj_unit_gen(c, 2, m), KD, key=("v", c, m))

            def add_outproj(fil, c):
                for tt4 in range(4):
                    for n2 in range(2):
                        fil.add(outproj_unit_gen(c, tt4, n2), PR)

            def attn_chunk(c, fil, after_hp=None):
                ntk = 4 * c + 4
                for hp in range(PR):
                    # program order is emission order: this head-pair's q/k
                    # projection units (hp-major FIFO order, so ensuring the
                    # k unit flushes the q unit too) must be emitted before
                    # its first scores matmul
                    fil.ensure(("qk", c, 1, hp))
                    ups = [
                        (psU0 if j == 0 else psU1).tile(
                            [HD + 1, 512], F32, name=f"up{j}", tag=f"u{j}")
                        for j in (0, 1)
                    ]
                    for tp in range(0, ntk, 2):
                        diag = tp >= 4 * c
                        r0 = P * (tp - 4 * c) if diag else 0
                        sps, ets = [], []
                        for i in (0, 1):
                            sps.append(psS.tile(
                                [P, 2, 512], F32, name="sp", tag="s"))
                            ets.append(epool.tile(
                                [P, 2, 512], BF16, name="et", tag="e"))
                        for i in (0, 1):
                            t = tp + i
                            tc_, t4 = t // 4, t % 4
                            ri = r0 + P * i if diag else 0
                            for j in (0, 1):
                                # j=0 rows 0-63, j=1 rows 64-127: disjoint
                                # row groups run concurrently on the PE
                                pb = 64 * j
                                nc.tensor.matmul(
                                    sps[i][:, j, ri:512],
                                    kt[tc_][pb:pb + 64, hp,
                                            t4 * P:(t4 + 1) * P],
                                    qt[c][pb:pb + 64, hp, ri:512],
                                    start=True,
                                    stop=True,
                                )
                        for i in (0, 1):
                            ri = r0 + P * i if diag else 0
                            nc.scalar.activation(
                                ets[i][:, :, ri:512], sps[i][:, :, ri:512],
                                mybir.ActivationFunctionType.Exp,
                                scale=float(1.0 / np.sqrt(HD)),
                            )
                            if diag:
                                # zero the upper triangle of the boundary
                                # 128-block with a static-mask DVE multiply
                                # (same mask for both heads)
                                nc.vector.tensor_tensor(
                                    out=ets[i][:, :, ri:ri + P],
                                    in0=ets[i][:, :, ri:ri + P],
                                    in1=tri[:],
                                    op=mybir.AluOpType.mult,
                                )
                        # filler between scores/exp and the dependent AV
                        # matmuls: the PE would otherwise stall here
                        fil.pump_point()
                        for i in (0, 1):
                            # producers of vv must be emitted before the AV
                            # matmuls that read them (emission order is
                            # program order)
                            t = tp + i
                            fil.ensure(("v", t // 4, t % 4))
                        for i in (0, 1):
                            t = tp + i
                            tc_, t4 = t // 4, t % 4
                            ri = r0 + P * i if diag else 0
                            for j in (0, 1):
                                nc.tensor.matmul(
                                    ups[j][:, ri:512],
                                    vv[tc_][:, t4, 2 * hp + j, :],
                                    ets[i][:, j, ri:512],
                                    start=(t == 0),
                                    stop=(t == ntk - 1),
                                )
                    # softmax normalization. The reciprocal of the denominator
                    # row is computed via the DMA-spread trick; the spread
                    # reads straight from PSUM so it runs concurrently with
                    # the U copy. The last head-pair of the last chunk skips
                    # the copy (multiply reads PSUM) -- nothing needs the
                    # banks afterwards.
                    lp = nc.allow_low_precision(
                        reason="bf16 softmax normalization; rel tol 2e-2")
                    lp.__enter__()
                    # both head-halves share one spread/recip/broadcast
                    # chain: the denominator rows are copied side by side
                    # first (the spread DMA waits only on these small
                    # copies), the full-U casts follow off the chain
                    dn = rpool.tile([HD + 1, 1024], BF16, name="dn",
                                    tag="dn")
                    for j in (0, 1):
                        nc.vector.tensor_copy(
                            dn[HD:HD + 1, j * 512:(j + 1) * 512],
                            ups[j][HD:HD + 1, :])
                    r4 = rpool.tile([64, 16], BF16, name="r4", tag="r4")
                    nc.sync.dma_start(r4[:], dn[HD:HD + 1, :])
                    uus = []
                    for j in (0, 1):
                        uu = upool.tile([HD, 512], BF16, name="uu",
                                        tag=f"uu{j}")
                        nc.vector.tensor_copy(uu[:], ups[j][0:HD, :])
                        uus.append(uu)
                    r4r = rpool.tile([64, 16], BF16, name="r4r", tag="r4r")
                    nc.vector.reciprocal(r4r[:], r4[:])
                    rb = rpool.tile([1, 1024], BF16, name="rb", tag="rb")
                    nc.sync.dma_start(rb[:], r4r[:])
                    bc = rpool.tile([64, 1024], BF16, name="bc", tag="bc")
                    nc.gpsimd.partition_broadcast(bc[:], rb[0:1, :])
                    nc.vector.tensor_tensor(
                        out=ot[c][hp][0:64, :], in0=uus[0][:],
                        in1=bc[:, 0:512], op=mybir.AluOpType.mult,
                    )
                    om = rpool.tile([64, 512], BF16, name="om", tag="om")
                    nc.vector.tensor_tensor(
                        out=om[:], in0=uus[1][:], in1=bc[:, 512:1024],
                        op=mybir.AluOpType.mult,
                    )
                    nc.sync.dma_start(ot[c][hp][64:128, :], om[:])
                    lp.__exit__(None, None, None)
                    if after_hp is not None:
                        after_hp(hp)

            # ---------------- schedule ----------------
            # chunk-0 Q/K projections run up front (attention needs them);
            # everything else flows through one continuous filler queue:
            #   during chunk 0: v0, qk1, v1
            #   during chunk 1: + qk2, v2
            #   during chunk 2: + qk3, op0
            #   during chunk 3: + v3, op1, op2 (+ op3 partials after hp2)
            # The adaptive pump quota leaves enough for the exp-bound late
            # chunks; the dependency-driven scheduler tolerates any slack.
            n_points = sum(4 * (2 * c + 2) for c in range(TC))
            fil = Filler(n_points)
            # hp0's q/k first, then v0 (hp0's AV ensures pull v0 through the
            # FIFO, so it must sit ahead of the later head-pairs' q/k)
            fil.add(proj_unit_gen(0, 0, 0), KD, key=("qk", 0, 0, 0))
            fil.add(proj_unit_gen(0, 1, 0), KD, key=("qk", 0, 1, 0))
            add_proj_v(fil, 0)
            for m in range(1, 4):
                fil.add(proj_unit_gen(0, 0, m), KD, key=("qk", 0, 0, m))
                fil.add(proj_unit_gen(0, 1, m), KD, key=("qk", 0, 1, m))
            add_proj_qk(fil, 1)
            add_proj_v(fil, 1)
            # emit chunk-0's projections as one dense block before its
            # attention: an early interleaved start stalls on weight arrival
            # and the resulting PE gaps re-throttle the HAM clock gate
            fil.ensure(("qk", 0, 1, 3))
            attn_chunk(0, fil)
            add_proj_qk(fil, 2)
            add_proj_v(fil, 2)
            attn_chunk(1, fil)
            add_proj_qk(fil, 3)
            add_outproj(fil, 0)
            attn_chunk(2, fil)
            add_proj_v(fil, 3)
            add_outproj(fil, 1)

            def after_hp3(hp):
                if hp == PR - 2:
                    for tt4 in range(4):
                        for n2 in range(2):
                            fil.add(outproj_partial_gen(TC - 1, tt4, n2),
                                    PR - 1)
                if hp == PR - 1:
                    # reserve: op2 emits right after the last AV group, so
                    # the PE grinds through it while the final normalization
                    # chain (cast/spread/recip/broadcast) runs on the other
                    # engines
                    add_outproj(fil, 2)

            attn_chunk(3, fil, after_hp=after_hp3)
            fil.drain()
            for tt4 in range(4):
                for n2 in range(2):
                    outproj_tail(TC - 1, tt4, n2)

    nc.compile()
    return nc


_NC_CACHE = None


def _get_nc():
    global _NC_CACHE
    if _NC_CACHE is None:
        _NC_CACHE = build_nc()
    return _NC_CACHE


def _shard_inputs(x, Wq, bq, Wk, bk, Wv, bv, Wp, bp):
    """Build the 8 per-core input maps."""
    bf16 = ml_dtypes.bfloat16
    x = np.asarray(x, dtype=np.float32)
    ca = np.ascontiguousarray
    in_maps = []
    for core in range(N_CORES):
        b, g = core // 2, core % 2
        cols = slice(g * DH, (g + 1) * DH)
        bq_g = np.asarray(bq[cols], np.float32).reshape(PR, P).T
        bk_g = np.asarray(bk[cols], np.float32).reshape(PR, P).T
        bv_g = np.broadcast_to(np.asarray(bv[cols], np.float32), (P, DH))
        if g == 0:
            bp_b = np.broadcast_to(np.asarray(bp, np.float32), (P, D))
        else:
            bp_b = np.zeros((P, D), np.float32)
        # permute into SBUF layouts host-side (contiguous DMA lines)
        xt_h = (x[b].T.reshape(KD, P, TC, 512)
                .transpose(1, 2, 0, 3).astype(bf16))       # [P, TC, KD, 512]
        wq_h = (np.asarray(Wq, np.float32)[:, cols]
                .reshape(KD, P, PR, P).transpose(1, 2, 0, 3).astype(bf16))
        wk_h = (np.asarray(Wk, np.float32)[:, cols]
                .reshape(KD, P, PR, P).transpose(1, 2, 0, 3).astype(bf16))
        wv_h = (np.asarray(Wv, np.float32)[:, cols]
                .reshape(KD, P, DH).transpose(1, 0, 2).astype(bf16))
        wp_h = (np.asarray(Wp, np.float32)[cols, :]
                .reshape(PR, P, D).transpose(1, 0, 2).astype(bf16))
        in_maps.append({
            "xt": ca(xt_h),
            "wq": ca(wq_h),
            "wk": ca(wk_h),
            "wv": ca(wv_h),
            "bq": ca(bq_g),
            "bk": ca(bk_g),
            "bvb": ca(bv_g),
            "wp": ca(wp_h),
            "bpb": ca(bp_b),
        })
    return in_maps


def run_sharded(inputs, trace=False):
    """Run on 8 cores; returns (full_output, BassKernelResults)."""
    nc = _get_nc()
    in_maps = _shard_inputs(**inputs)
    res = run_bass_kernel_spmd(
        nc, in_maps, core_ids=list(range(N_CORES)), trace=trace
    )
    out = np.empty((B, T, D), np.float32)
    for b in range(B):
        out[b] = res.results[2 * b]["y"] + res.results[2 * b + 1]["y"]
    return out, res


def kernel(**inputs) -> np.ndarray:
    out, _ = run_sharded(inputs)
    return out
